# revision 16
# baseline (speedup 1.0000x reference)
"""Trainium2 Bass kernel for capsule dynamic routing (nn_Capsule) — v4.

Reference (per batch item b):
    u = x_b @ W; logits = 0
    for i in 4:
        c = softmax(logits, axis=capsule)
        t_j = sum_s c[s,j] * u[s, j*64:(j+1)*64]; v = squash(t)
        if i < 3: logits[s,j] += u[s, jblk] . v_j

Never materializes u (linearity):
    y_j   = sum_s c[s,j] x_s            y-GEMM   (c stationary, col-tiled)
    t     = W^T y^T                     t-GEMM   (w16 stationary per-slice)
    P^T   = Vblk^T W^T                  P-GEMM   (vblk stationary, block-diag)
    upd^T = P^T X                       upd-GEMM (P slices stationary, col-tiled)

v4 vs v3 (230us): trace showed PE active only 53%, HAM cold 40% of span.
  - queue discipline: ALL bulk input loads ride the scalar HWDGE queue;
    the sync queue carries only tiny consts then mid-iteration
    transposes.  (v3 split loads across both queues, so iteration-0/1
    transposes sat 40+us behind xt16 loads -> PE starved -> HAM cold.)
  - load order w16 -> x16 (per b,half chunks) -> wt16 -> xt16 (per
    b,hcq chunks), matching first-use order; iteration-0 y-GEMM is
    emitted batch-major so it consumes x16 chunks as they land.
  - per-stage emission reordered so every stage-bridging evac/
    transpose/softmax hides under another block's matmuls:
      y: half-outer (iters>=1) with per-(half,g) evac+transpose;
      upd: per-g softmax tail emitted between the two hcq1 g-blocks;
      t: per-oc block-diag extract emitted right after that oc's last
      accumulation matmul.
  - transposes alternate sync/scalar queues once loads are done.
  - warm fillers only where a real >3us PE gap is unavoidable
    (pre-stage), rhs=zeros so they never dep on input DMAs.

HW lessons kept from v3:
  - DVE copy PSUM(f32)->SBUF(f16) kills the device; PSUM->f16 casts go
    through ScalarE activation(Copy).
  - matmul start=True lazily zeroes the whole 2KB PSUM bank: accumulation
    groups must own a (partition-range x bank) region exclusively;
    partition-disjoint groups interleave with skip_group_check=True;
    column-disjoint writes into one bank are fine after the first
    start=True (has_written is per-element).
  - squash rsqrt on DVE (bitcast seed + 2 Newton steps); ScalarE runs
    only Copy+Exp -> exactly ONE ACT table load for the whole kernel.
  - nc.vector.memset on f16 tiles is unreliable: constants come from host.
"""
import numpy as np
from contextlib import ExitStack

import concourse.bass as bass
import concourse.bacc as bacc
import concourse.tile as tile
from concourse import mybir
from concourse.bass_utils import run_bass_kernel_spmd

f16 = mybir.dt.float16
f32 = mybir.dt.float32
i32 = mybir.dt.int32
COPY = mybir.ActivationFunctionType.Copy
EXP = mybir.ActivationFunctionType.Exp
MULT = mybir.AluOpType.mult
SUB = mybir.AluOpType.subtract
ADD = mybir.AluOpType.add
SHR = mybir.AluOpType.logical_shift_right

S, B, H = 512, 64, 1024
NCAP, DCAP = 16, 64
ROUTINGS = 4
N_CORES = 8
BL = B // N_CORES          # 8 batch items per core
SC = S // 128              # 4 s-chunks
HC = H // 128              # 8 h-chunks
OC = H // 128              # 8 o-chunks (o = NCAP*DCAP = 1024)
MAGIC = 0x5EF759DF         # rsqrt seed for h = s/2: 0x5f3759df - (1<<22)


def _act_copy(nc, out, in_):
    nc.scalar.activation(out=out, in_=in_, func=COPY, scale=1.0, alpha=0.0)


def _build_kernel(tc, out_d, x_d, xt_d, w_d, wt_d, c0_d, logits_d, vblk_d,
                  ones2_d, o2t_d, zeros_d, magic_d):
    nc = tc.nc
    ctx = ExitStack()
    const = ctx.enter_context(tc.tile_pool(name="const", bufs=1))
    work = ctx.enter_context(tc.tile_pool(name="work", bufs=1))
    small = ctx.enter_context(tc.tile_pool(name="small", bufs=2))
    ps_big = ctx.enter_context(tc.tile_pool(name="ps_big", bufs=2,
                                            space="PSUM"))
    ps_u = ctx.enter_context(tc.tile_pool(name="ps_u", bufs=2, space="PSUM"))
    ps_sm = ctx.enter_context(tc.tile_pool(name="ps_sm", bufs=2,
                                           space="PSUM"))

    # ---------- persistent tensors ----------
    x16 = const.tile([128, BL, SC, 1024], f16)      # (s_loc, b, sc, h)
    xt16 = const.tile([128, BL, HC, 512], f16)      # (h_loc, b, hc, s)
    w16 = const.tile([128, HC, 1024], f16)          # (h_loc, hc, o)
    wt16 = const.tile([128, OC, 1024], f16)         # (o_loc, oc, h)
    c16 = const.tile([128, SC, BL, 32], f16)        # coeffs, cols 16-31 zero
    logits = const.tile([128, SC, BL, NCAP], f32)
    vblk = const.tile([128, OC, 128], f16)          # block-diag v, bj dense
    ones2 = const.tile([128, 2], f16)               # [[1;0],[0;1]] halves
    o2t = const.tile([2, 128], f32)                 # broadcast helper
    zeros = const.tile([128, 128], f16)             # zero-weight psum fill
    magic = const.tile([2, 1], i32)                 # rsqrt seed constant

    # ---------- loads ----------
    # ALL DMAs (loads + transposes) ride the sync (SP) queue: HWDGE DMA
    # issue blocks the issuing engine for ~0.5-1.5us per call, so the
    # scalar (ACT) engine must never issue DMAs or its evacs stall.
    # Loads are big per-b contiguous chunks in first-use order.
    nc.sync.dma_start(out=zeros[:], in_=zeros_d[:])
    nc.sync.dma_start(out=ones2[:], in_=ones2_d[:])
    nc.sync.dma_start(out=o2t[:], in_=o2t_d[:])
    nc.sync.dma_start(out=magic[:], in_=magic_d[:])
    nc.sync.dma_start(out=c16[:], in_=c0_d[:])
    nc.sync.dma_start(out=logits[:], in_=logits_d[:])
    nc.sync.dma_start(out=vblk[:], in_=vblk_d[:])
    # x16 b0-3 first so iteration-0 y-GEMM starts ASAP; w16 before x b4-7
    # (t-GEMM needs it right after y); wt16 before xt16 (P before upd).
    for b in range(4):
        nc.sync.dma_start(out=x16[:, b], in_=x_d[:, b])
    nc.sync.dma_start(out=w16[:], in_=w_d[:])
    for b in range(4, BL):
        nc.sync.dma_start(out=x16[:, b], in_=x_d[:, b])
    nc.sync.dma_start(out=wt16[:], in_=wt_d[:])
    for b in range(BL):
        nc.sync.dma_start(out=xt16[:, b], in_=xt_d[:, b])

    def _warm(ps_tile, n, big=False):
        # zero-weight matmuls into a psum tile whose next real producer
        # begins with start=True (which wipes the bank): pure HAM fuel
        # that runs during the dependency-wait gap before the stage.
        # big=True streams x16 (N=512, 215ns each) for post-load fillers;
        # otherwise rhs=zeros so fillers never wait on input DMAs.
        for k in range(n):
            if big:
                nc.tensor.matmul(ps_tile[:, 0:512], zeros[:],
                                 x16[:, 0, 0, 0:512],
                                 start=(k == 0), stop=False,
                                 skip_group_check=True)
            else:
                nc.tensor.matmul(ps_tile[:, 0:128], zeros[:], zeros[:],
                                 start=(k == 0), stop=False,
                                 skip_group_check=True)

    def _tq(it):
        # Tile serializes a DMA_TRANSPOSE against ALL prior DMAs on its
        # queue (deadlock guard), so while input loads are still in
        # flight on sync (iters 0-1), transposes ride the scalar queue
        # (DMA-free).  Later iterations use sync so ScalarE's evac/exp
        # chain is not blocked by transpose issue time.
        return nc.scalar if it < 2 else nc.sync

    for it in range(ROUTINGS):
        last = it == ROUTINGS - 1

        # ---------- y = C^T X ----------
        y_ps = [ps_big.tile([128, 1024], f32, tag="big", name=f"y{it}_{g}")
                for g in range(2)]
        if it == 0:
            _warm(y_ps[0], 16)
            import os
            if os.environ.get("DIAG_FILL"):
                _warm(y_ps[0], 300, big=True)
        else:
            # cover the tail-g1 softmax chain + keep HAM warm
            _warm(y_ps[0], 8, big=True)
        y_sb = work.tile([128, 2, 1024], f16, tag="y_sb")
        yt = work.tile([128, HC, 256], f16, tag="yt")

        def _y_mm(g, b_, half, sc):
            b = 4 * g + b_
            hs = slice(512 * half, 512 * half + 512)
            nc.tensor.matmul(
                y_ps[g][32 * b_:32 * b_ + 32, hs],
                c16[:, sc, b, :],
                x16[:, b, sc, hs],
                start=(sc == 0), stop=(sc == SC - 1),
                skip_group_check=True,
                tile_position=(0, 32 * b_))

        def _y_evac(g, half):
            hs = slice(512 * half, 512 * half + 512)
            _act_copy(nc, y_sb[:, g, hs], y_ps[g][:, hs])
            _tq(it).dma_start_transpose(
                yt[:, 4 * half:4 * half + 4, 128 * g:128 * g + 128],
                y_sb[:, g, hs])

        if it == 0:
            # batch-major: track x16 arrival order b0..b7
            for g in range(2):
                for b_ in range(4):
                    for half in range(2):
                        for sc in range(SC):
                            _y_mm(g, b_, half, sc)
                for half in range(2):
                    _y_evac(g, half)
        else:
            # g-outer: y-g0 runs while upd tails s2/s3 finish; y-g1's
            # c16 is ready by the time y-g0's matmuls complete
            for g in range(2):
                for half in range(2):
                    for b_ in range(4):
                        for sc in range(SC):
                            _y_mm(g, b_, half, sc)
                    _y_evac(g, half)

        # ---------- t = W^T y^T, two hc passes ----------
        yt_dense = yt.rearrange("p hc (g b_ jp) -> p hc g b_ jp", g=2, jp=32)
        t_ps = ps_big.tile([128, 1024], f32, tag="big", name=f"t{it}")
        _warm(t_ps, 2 if it == 0 else 4, big=(it > 0))
        for hcq in range(2):
            for oc in range(OC):
                for hc in range(4 * hcq, 4 * hcq + 4):
                    nc.tensor.matmul(
                        t_ps[:, oc * 128:oc * 128 + 128],
                        w16[:, hc, oc * 128:oc * 128 + 128],
                        yt_dense[:, hc, :, :, 0:16],
                        start=(hcq == 0 and hc == 0 and oc % 4 == 0),
                        stop=(hcq == 1 and hc == 7),
                        skip_group_check=True)

        # ---------- extract block-diag: t_sb (e*64+d, b*8+oc) ----------
        t_sb = small.tile([128, 64], f32, tag="t_sb")
        tv = t_ps.rearrange("p (oc b j) -> p oc b j", oc=OC, b=BL)
        dv = t_sb.rearrange("p (b oc) -> p b oc", oc=OC)
        for oc in range(OC):
            nc.vector.tensor_copy(dv[0:64, :, oc], tv[0:64, oc, :, 2 * oc])
            _act_copy(nc, dv[64:128, :, oc], tv[64:128, oc, :, 2 * oc + 1])

        # ---------- squash: rs = rsqrt(sum_d t^2 + eps) on DVE ----------
        t2 = small.tile([128, 64], f16, tag="t2")
        nc.vector.tensor_mul(t2[:], t_sb[:], t_sb[:])
        sq_ps = ps_sm.tile([2, 512], f32, tag="sm", name=f"sq{it}")
        nc.tensor.matmul(sq_ps[:, 0:64], ones2[:], t2[:])
        h_sb = small.tile([2, 64], f32, tag="h_sb")
        nc.vector.tensor_scalar(out=h_sb[:], in0=sq_ps[:, 0:64],
                                scalar1=0.5, scalar2=5e-8, op0=MULT, op1=ADD)
        ri = small.tile([2, 64], i32, tag="ri")
        nc.vector.tensor_scalar(out=ri[:], in0=h_sb.bitcast(i32),
                                scalar1=1, scalar2=None, op0=SHR)
        r0 = small.tile([2, 64], f32, tag="r0")
        nc.vector.tensor_tensor(out=r0.bitcast(i32),
                                in0=magic.broadcast_to([2, 64]),
                                in1=ri[:], op=SUB)
        # Newton x1 with sign fold: rs = (h*r0*r0 - 1.5) * r0 = -rsqrt
        # approx (0.2% err, fine at 2e-2 tol); o2t carries -1 so the
        # broadcast flips the sign back.
        rr = small.tile([2, 64], f32, tag="rr")
        rs = small.tile([2, 64], f32, tag="rs")
        nc.vector.tensor_mul(rr[:], r0[:], r0[:])
        nc.vector.tensor_mul(rr[:], rr[:], h_sb[:])
        nc.vector.scalar_tensor_tensor(out=rs[:], in0=rr[:], scalar=1.5,
                                       in1=r0[:], op0=SUB, op1=MULT)
        # broadcast -rs (2,64) -> (128,64): bc[p,n] = -rs[p//64, n]
        bc_ps = ps_sm.tile([128, 512], f32, tag="sm", name=f"bc{it}")
        nc.tensor.matmul(bc_ps[:, 0:64], o2t[:], rs[:])

        if last:
            v32 = small.tile([128, 64], f32, tag="v32")
            nc.vector.tensor_mul(v32[:], t_sb[:], bc_ps[:, 0:64])
            # raw dump; host reorders (out[b,2oc+e,d] = v32[e*64+d, b*8+oc])
            nc.sync.dma_start(out=out_d, in_=v32[:])
            break

        # ---------- scatter v = t*bc into block-diag vblk (fused) -------
        # vblk[e*64+d, oc, b*16+2oc+e] = t_sb[e*64+d, b*8+oc] * bc[...]
        pitch = vblk[:].ap[0][0]
        for e in range(2):
            dst = bass.AP(tensor=vblk[:].tensor,
                          offset=64 * e * pitch + e,
                          ap=[[pitch, 64], [130, OC], [16, BL]])
            nc.vector.tensor_tensor(
                out=dst,
                in0=t_sb[64 * e:64 * e + 64, :].rearrange(
                    "p (b o) -> p o b", o=OC),
                in1=bc_ps[64 * e:64 * e + 64, 0:64].rearrange(
                    "p (b o) -> p o b", o=OC),
                op=MULT)

        # ---------- P^T = Vblk^T W^T, h-halves pipelined ----------
        pT_ps = ps_big.tile([128, 1024], f32, tag="big", name=f"pT{it}")
        _warm(pT_ps, 3 if it == 0 else 8, big=(it > 0))
        pT_sb = work.tile([128, 1024], f16, tag="pT_sb")
        p_sb = work.tile([128, HC, 128], f16, tag="p_sb")
        for half in range(2):
            hs = slice(512 * half, 512 * half + 512)
            for oc in range(OC):
                nc.tensor.matmul(
                    pT_ps[:, hs],
                    vblk[:, oc, :],
                    wt16[:, oc, hs],
                    start=(oc == 0), stop=(oc == OC - 1),
                    skip_group_check=True)
            _act_copy(nc, pT_sb[:, hs], pT_ps[:, hs])
            _tq(it).dma_start_transpose(p_sb[:, 4 * half:4 * half + 4, :],
                                        pT_sb[:, hs])

        # ---------- upd^T = P^T X (col-tiled per batch) ----------
        # 2 groups of 4 batches (4-way col-tiled); each group's softmax
        # tail chain hides under the other group's matmuls / next y-g0.
        u_ps = [ps_u.tile([128, 512], f32, tag="u", name=f"u{it}_{g}")
                for g in range(2)]
        u_sb = work.tile([128, 2, 512], f16, tag="u_sb")
        ut = work.tile([128, 2, SC, 128], f16, tag="ut")
        _warm(u_ps[0], 3 if it == 0 else 6, big=(it > 0))
        for g in range(2):
            nc.tensor.matmul(u_ps[g][:], zeros[:], x16[:, 0, 0, 0:512],
                             start=True, stop=False, skip_group_check=True)

        def _upd_tail(g):
            # evac, transpose, logits add, softmax -> c16 for group g
            _act_copy(nc, u_sb[:, g, :], u_ps[g][:])
            _tq(it).dma_start_transpose(ut[:, g], u_sb[:, g, :])
            srcu = ut[:, g].rearrange("p sc (b_ jp) -> p sc b_ jp", jp=32)
            nc.vector.tensor_add(
                logits[:, :, 4 * g:4 * g + 4, :],
                logits[:, :, 4 * g:4 * g + 4, :], srcu[:, :, :, 0:16])
            ex = small.tile([128, SC, 4, NCAP], f32, tag="ex")
            nc.scalar.activation(out=ex[:],
                                 in_=logits[:, :, 4 * g:4 * g + 4, :],
                                 func=EXP, scale=1.0, alpha=0.0)
            sm = small.tile([128, SC, 4, 1], f32, tag="sm")
            nc.vector.reduce_sum(sm[:], ex[:], axis=mybir.AxisListType.X)
            rc = small.tile([128, SC, 4, 1], f32, tag="rc")
            nc.vector.reciprocal(rc[:], sm[:])
            nc.vector.tensor_mul(c16[:, :, 4 * g:4 * g + 4, 0:16], ex[:],
                                 rc.broadcast_to([128, SC, 4, NCAP]))

        for g in range(2):
            for hc in range(HC):
                for b_ in range(4):
                    b = 4 * g + b_
                    nc.tensor.matmul(
                        u_ps[g][32 * b_:32 * b_ + 16, :],
                        p_sb[:, hc, 16 * b:16 * b + 16],
                        xt16[:, b, hc, :],
                        start=False, stop=(hc == 7),
                        skip_group_check=True,
                        tile_position=(0, 32 * b_))
            _upd_tail(g)
    ctx.close()


_CACHE = {}


def _host_consts():
    c0 = np.zeros((128, SC, BL, 32), np.float16)
    c0[:, :, :, 0:NCAP] = np.float16(1.0 / NCAP)
    logi = np.zeros((128, SC, BL, NCAP), np.float32)
    vblk0 = np.zeros((128, OC, 128), np.float16)
    ones2 = np.zeros((128, 2), np.float16)
    ones2[0:64, 0] = 1.0
    ones2[64:128, 1] = 1.0
    o2t = np.zeros((2, 128), np.float32)
    o2t[0, 0:64] = -1.0          # sign-fix for the single-Newton rsqrt
    o2t[1, 64:128] = -1.0
    zeros = np.zeros((128, 128), np.float16)
    magic = np.full((2, 1), MAGIC, np.int32)
    return {"c0i": c0, "logi": logi, "vblki": vblk0, "ones2": ones2,
            "o2t": o2t, "zeros": zeros, "magic": magic}


def _get_nc():
    if "nc" not in _CACHE:
        nc = bacc.Bacc("TRN2", target_bir_lowering=False, debug=False)
        x_d = nc.dram_tensor("x16", [128, BL, SC, 1024], f16,
                             kind="ExternalInput")
        xt_d = nc.dram_tensor("xt16", [128, BL, HC, 512], f16,
                              kind="ExternalInput")
        w_d = nc.dram_tensor("w16", [128, HC, 1024], f16,
                             kind="ExternalInput")
        wt_d = nc.dram_tensor("wt16", [128, OC, 1024], f16,
                              kind="ExternalInput")
        c0_d = nc.dram_tensor("c0i", [128, SC, BL, 32], f16,
                              kind="ExternalInput")
        logits_d = nc.dram_tensor("logi", [128, SC, BL, NCAP], f32,
                                  kind="ExternalInput")
        vblk_d = nc.dram_tensor("vblki", [128, OC, 128], f16,
                                kind="ExternalInput")
        ones2_d = nc.dram_tensor("ones2", [128, 2], f16, kind="ExternalInput")
        o2t_d = nc.dram_tensor("o2t", [2, 128], f32, kind="ExternalInput")
        zeros_d = nc.dram_tensor("zeros", [128, 128], f16,
                                 kind="ExternalInput")
        magic_d = nc.dram_tensor("magic", [2, 1], i32, kind="ExternalInput")
        out_d = nc.dram_tensor("out", [128, 64], f32, kind="ExternalOutput")
        with tile.TileContext(nc) as tc:
            _build_kernel(tc, out_d.ap(), x_d.ap(), xt_d.ap(), w_d.ap(),
                          wt_d.ap(), c0_d.ap(), logits_d.ap(), vblk_d.ap(),
                          ones2_d.ap(), o2t_d.ap(), zeros_d.ap(),
                          magic_d.ap())
        nc.compile()
        _CACHE["nc"] = nc
    return _CACHE["nc"]


def kernel(inputs: np.ndarray, W: np.ndarray, _trace: bool = False):
    """inputs: (512, 64, 1024) f32; W: (1, 1024, 1024) f32.
    Returns (64, 16, 64) f32."""
    nc = _get_nc()
    consts = _host_consts()
    w0 = W[0].astype(np.float16)
    w16h = np.ascontiguousarray(
        w0.reshape(HC, 128, 1024).transpose(1, 0, 2))
    wt16h = np.ascontiguousarray(
        w0.reshape(1024, OC, 128).transpose(2, 1, 0))
    x16f = inputs.astype(np.float16)              # (S, B, H)
    in_maps = []
    for c in range(N_CORES):
        xs = x16f[:, c * BL:(c + 1) * BL, :]      # (S, BL, H)
        x16h = np.ascontiguousarray(
            xs.reshape(SC, 128, BL, H).transpose(1, 2, 0, 3))
        xt16h = np.ascontiguousarray(
            xs.reshape(S, BL, HC, 128).transpose(3, 1, 2, 0))
        m = {"x16": x16h, "xt16": xt16h, "w16": w16h, "wt16": wt16h}
        m.update(consts)
        in_maps.append(m)
    kw = {}
    if _trace:
        kw = dict(trace=True, trace_cores=[0], stitch_traces=False)
    res = run_bass_kernel_spmd(nc, in_maps, core_ids=list(range(N_CORES)),
                               **kw)
    outs = []
    for c in range(N_CORES):
        v = res.results[c]["out"]          # (128=e*64+d, 64=b*8+oc)
        outs.append(v.reshape(2, 64, BL, 8).transpose(2, 3, 0, 1)
                     .reshape(BL, NCAP, DCAP))
    out = np.concatenate(outs, axis=0)
    if _trace:
        return out.astype(np.float32), res
    return out.astype(np.float32)


# revision 19
# speedup vs baseline: 1.1676x; 1.1676x over previous
"""Trainium2 Bass kernel for capsule dynamic routing (nn_Capsule) — v4.

Reference (per batch item b):
    u = x_b @ W; logits = 0
    for i in 4:
        c = softmax(logits, axis=capsule)
        t_j = sum_s c[s,j] * u[s, j*64:(j+1)*64]; v = squash(t)
        if i < 3: logits[s,j] += u[s, jblk] . v_j

Never materializes u (linearity):
    y_j   = sum_s c[s,j] x_s            y-GEMM   (c stationary, col-tiled)
    t     = W^T y^T                     t-GEMM   (w16 stationary per-slice)
    P^T   = Vblk^T W^T                  P-GEMM   (vblk stationary, block-diag)
    upd^T = P^T X                       upd-GEMM (P slices stationary, col-tiled)

v4 vs v3 (230us): trace showed PE active only 53%, HAM cold 40% of span.
  - queue discipline: ALL bulk input loads ride the scalar HWDGE queue;
    the sync queue carries only tiny consts then mid-iteration
    transposes.  (v3 split loads across both queues, so iteration-0/1
    transposes sat 40+us behind xt16 loads -> PE starved -> HAM cold.)
  - load order w16 -> x16 (per b,half chunks) -> wt16 -> xt16 (per
    b,hcq chunks), matching first-use order; iteration-0 y-GEMM is
    emitted batch-major so it consumes x16 chunks as they land.
  - per-stage emission reordered so every stage-bridging evac/
    transpose/softmax hides under another block's matmuls:
      y: half-outer (iters>=1) with per-(half,g) evac+transpose;
      upd: per-g softmax tail emitted between the two hcq1 g-blocks;
      t: per-oc block-diag extract emitted right after that oc's last
      accumulation matmul.
  - transposes alternate sync/scalar queues once loads are done.
  - warm fillers only where a real >3us PE gap is unavoidable
    (pre-stage), rhs=zeros so they never dep on input DMAs.

HW lessons kept from v3:
  - DVE copy PSUM(f32)->SBUF(f16) kills the device; PSUM->f16 casts go
    through ScalarE activation(Copy).
  - matmul start=True lazily zeroes the whole 2KB PSUM bank: accumulation
    groups must own a (partition-range x bank) region exclusively;
    partition-disjoint groups interleave with skip_group_check=True;
    column-disjoint writes into one bank are fine after the first
    start=True (has_written is per-element).
  - squash rsqrt on DVE (bitcast seed + 2 Newton steps); ScalarE runs
    only Copy+Exp -> exactly ONE ACT table load for the whole kernel.
  - nc.vector.memset on f16 tiles is unreliable: constants come from host.
"""
import numpy as np
from contextlib import ExitStack

import concourse.bass as bass
import concourse.bacc as bacc
import concourse.tile as tile
from concourse import mybir
from concourse.bass_utils import run_bass_kernel_spmd

f16 = mybir.dt.float16
f32 = mybir.dt.float32
i32 = mybir.dt.int32
COPY = mybir.ActivationFunctionType.Copy
EXP = mybir.ActivationFunctionType.Exp
MULT = mybir.AluOpType.mult
SUB = mybir.AluOpType.subtract
ADD = mybir.AluOpType.add
SHR = mybir.AluOpType.logical_shift_right

S, B, H = 512, 64, 1024
NCAP, DCAP = 16, 64
ROUTINGS = 4
N_CORES = 8
BL = B // N_CORES          # 8 batch items per core
SC = S // 128              # 4 s-chunks
HC = H // 128              # 8 h-chunks
OC = H // 128              # 8 o-chunks (o = NCAP*DCAP = 1024)
MAGIC = 0x5EF759DF         # rsqrt seed for h = s/2: 0x5f3759df - (1<<22)


def _act_copy(nc, out, in_):
    nc.scalar.activation(out=out, in_=in_, func=COPY, scale=1.0, alpha=0.0)


def _build_kernel(tc, out_d, x_d, xt_d, w_d, wt_d, c0_d, logits_d, vblk_d,
                  ones2_d, o2t_d, zeros_d, magic_d):
    nc = tc.nc
    ctx = ExitStack()
    const = ctx.enter_context(tc.tile_pool(name="const", bufs=1))
    work = ctx.enter_context(tc.tile_pool(name="work", bufs=1))
    small = ctx.enter_context(tc.tile_pool(name="small", bufs=2))
    ps_big = ctx.enter_context(tc.tile_pool(name="ps_big", bufs=2,
                                            space="PSUM"))
    ps_u = ctx.enter_context(tc.tile_pool(name="ps_u", bufs=2, space="PSUM"))
    ps_sm = ctx.enter_context(tc.tile_pool(name="ps_sm", bufs=2,
                                           space="PSUM"))

    # ---------- persistent tensors ----------
    x16 = const.tile([128, BL, SC, 1024], f16)      # (s_loc, b, sc, h)
    xt16 = const.tile([128, BL, HC, 512], f16)      # (h_loc, b, hc, s)
    w16 = const.tile([128, HC, 1024], f16)          # (h_loc, hc, o)
    wt16 = const.tile([128, OC, 1024], f16)         # (o_loc, oc, h)
    c16 = const.tile([128, SC, BL, 32], f16)        # coeffs, cols 16-31 zero
    logits = const.tile([128, SC, BL, NCAP], f32)
    vblk = const.tile([128, OC, 128], f16)          # block-diag v, bj dense
    ones2 = const.tile([128, 2], f16)               # [[1;0],[0;1]] halves
    o2t = const.tile([2, 128], f32)                 # broadcast helper
    zeros = const.tile([128, 128], f16)             # zero-weight psum fill
    magic = const.tile([2, 1], i32)                 # rsqrt seed constant

    # ---------- loads ----------
    # ALL DMAs (loads + transposes) ride the sync (SP) queue: HWDGE DMA
    # issue blocks the issuing engine for ~0.5-1.5us per call, so the
    # scalar (ACT) engine must never issue DMAs or its evacs stall.
    # Loads are big per-b contiguous chunks in first-use order.
    nc.sync.dma_start(out=zeros[:], in_=zeros_d[:])
    nc.sync.dma_start(out=ones2[:], in_=ones2_d[:])
    nc.sync.dma_start(out=o2t[:], in_=o2t_d[:])
    nc.sync.dma_start(out=magic[:], in_=magic_d[:])
    nc.sync.dma_start(out=c16[:], in_=c0_d[:])
    nc.sync.dma_start(out=logits[:], in_=logits_d[:])
    nc.sync.dma_start(out=vblk[:], in_=vblk_d[:])
    # x16 b0-3 first so iteration-0 y-GEMM starts ASAP; w16 before x b4-7
    # (t-GEMM needs it right after y); wt16 before xt16 (P before upd).
    for b in range(4):
        nc.sync.dma_start(out=x16[:, b], in_=x_d[:, b])
    nc.sync.dma_start(out=w16[:], in_=w_d[:])
    for b in range(4, BL):
        nc.sync.dma_start(out=x16[:, b], in_=x_d[:, b])
    nc.sync.dma_start(out=wt16[:], in_=wt_d[:])
    for b in range(BL):
        nc.sync.dma_start(out=xt16[:, b], in_=xt_d[:, b])

    def _warm(ps_tile, n, big=False):
        # zero-weight matmuls into a psum tile whose next real producer
        # begins with start=True (which wipes the bank): pure HAM fuel
        # that runs during the dependency-wait gap before the stage.
        # big=True streams x16 (N=512, 215ns each) for post-load fillers;
        # otherwise rhs=zeros so fillers never wait on input DMAs.
        for k in range(n):
            if big:
                nc.tensor.matmul(ps_tile[:, 0:512], zeros[:],
                                 x16[:, 0, 0, 0:512],
                                 start=(k == 0), stop=False,
                                 skip_group_check=True)
            else:
                nc.tensor.matmul(ps_tile[:, 0:128], zeros[:], zeros[:],
                                 start=(k == 0), stop=False,
                                 skip_group_check=True)

    def _tq(it):
        # Tile serializes a DMA_TRANSPOSE against ALL prior DMAs on its
        # queue (deadlock guard), so while input loads are still in
        # flight on sync (iters 0-1), transposes ride the scalar queue
        # (DMA-free).  Later iterations use sync so ScalarE's evac/exp
        # chain is not blocked by transpose issue time.
        return nc.scalar if it < 2 else nc.sync

    for it in range(ROUTINGS):
        last = it == ROUTINGS - 1

        # ---------- y = C^T X ----------
        y_ps = [ps_big.tile([128, 1024], f32, tag="big", name=f"y{it}_{g}")
                for g in range(2)]
        if it == 0:
            _warm(y_ps[0], 16)
            import os
            if os.environ.get("DIAG_FILL"):
                _warm(y_ps[0], 300, big=True)
        else:
            # cover the tail-g1 softmax chain + keep HAM warm
            _warm(y_ps[0], 8, big=True)
        y_sb = work.tile([128, 2, 1024], f16, tag="y_sb")
        yt = work.tile([128, HC, 256], f16, tag="yt")

        def _y_mm(g, b_, half, sc):
            b = 4 * g + b_
            hs = slice(512 * half, 512 * half + 512)
            nc.tensor.matmul(
                y_ps[g][32 * b_:32 * b_ + 32, hs],
                c16[:, sc, b, :],
                x16[:, b, sc, hs],
                start=(sc == 0), stop=(sc == SC - 1),
                skip_group_check=True,
                tile_position=(0, 32 * b_))

        def _y_evac(g, half):
            hs = slice(512 * half, 512 * half + 512)
            _act_copy(nc, y_sb[:, g, hs], y_ps[g][:, hs])
            _tq(it).dma_start_transpose(
                yt[:, 4 * half:4 * half + 4, 128 * g:128 * g + 128],
                y_sb[:, g, hs])

        if it == 0:
            # batch-major: track x16 arrival order b0..b7
            for g in range(2):
                for b_ in range(4):
                    for half in range(2):
                        for sc in range(SC):
                            _y_mm(g, b_, half, sc)
                for half in range(2):
                    _y_evac(g, half)
        else:
            # g-outer: y-g0 runs while upd tails s2/s3 finish; y-g1's
            # c16 is ready by the time y-g0's matmuls complete
            for g in range(2):
                for half in range(2):
                    for b_ in range(4):
                        for sc in range(SC):
                            _y_mm(g, b_, half, sc)
                    _y_evac(g, half)

        # ---------- t = W^T y^T, two hc passes ----------
        yt_dense = yt.rearrange("p hc (g b_ jp) -> p hc g b_ jp", g=2, jp=32)
        t_ps = ps_big.tile([128, 1024], f32, tag="big", name=f"t{it}")
        _warm(t_ps, 2 if it == 0 else 4, big=(it > 0))
        for hcq in range(2):
            for oc in range(OC):
                for hc in range(4 * hcq, 4 * hcq + 4):
                    nc.tensor.matmul(
                        t_ps[:, oc * 128:oc * 128 + 128],
                        w16[:, hc, oc * 128:oc * 128 + 128],
                        yt_dense[:, hc, :, :, 0:16],
                        start=(hcq == 0 and hc == 0 and oc % 4 == 0),
                        stop=(hcq == 1 and hc == 7),
                        skip_group_check=True)
            if hcq == 0 and it > 0:
                # bridge the yt-h1 transpose wait before the hcq1 pass
                fb = ps_sm.tile([128, 512], f32, tag="sm", name=f"tf{it}")
                _warm(fb, 5, big=True)

        # ---------- extract block-diag: t_sb (e*64+d, b*8+oc) ----------
        # t_ps col for (oc, b, j=2oc+e) = oc*130 + b*16 + e: linear in
        # (oc, b), so one strided copy per e-half (DVE + ScalarE in
        # parallel) replaces 16 tiny per-oc copies.
        t_sb = small.tile([128, 64], f32, tag="t_sb")
        pps = t_ps[:].ap[0][0]
        psb = t_sb[:].ap[0][0]
        for e in range(2):
            srcd = bass.AP(tensor=t_ps[:].tensor, offset=64 * e * pps + e,
                           ap=[[pps, 64], [130, OC], [16, BL]])
            dstd = bass.AP(tensor=t_sb[:].tensor, offset=64 * e * psb,
                           ap=[[psb, 64], [1, OC], [8, BL]])
            if e == 0:
                nc.vector.tensor_copy(dstd, srcd)
            else:
                _act_copy(nc, dstd, srcd)

        # ---------- squash: rs = rsqrt(sum_d t^2 + eps) on DVE ----------
        t2 = small.tile([128, 64], f16, tag="t2")
        nc.vector.tensor_mul(t2[:], t_sb[:], t_sb[:])
        sq_ps = ps_sm.tile([2, 512], f32, tag="sm", name=f"sq{it}")
        nc.tensor.matmul(sq_ps[:, 0:64], ones2[:], t2[:])
        h_sb = small.tile([2, 64], f32, tag="h_sb")
        nc.vector.tensor_scalar(out=h_sb[:], in0=sq_ps[:, 0:64],
                                scalar1=0.5, scalar2=5e-8, op0=MULT, op1=ADD)
        ri = small.tile([2, 64], i32, tag="ri")
        nc.vector.tensor_scalar(out=ri[:], in0=h_sb.bitcast(i32),
                                scalar1=1, scalar2=None, op0=SHR)
        r0 = small.tile([2, 64], f32, tag="r0")
        nc.vector.tensor_tensor(out=r0.bitcast(i32),
                                in0=magic.broadcast_to([2, 64]),
                                in1=ri[:], op=SUB)
        # Newton x1 with sign fold: rs = (h*r0*r0 - 1.5) * r0 = -rsqrt
        # approx (0.2% err, fine at 2e-2 tol); o2t carries -1 so the
        # broadcast flips the sign back.
        rr = small.tile([2, 64], f32, tag="rr")
        rs = small.tile([2, 64], f32, tag="rs")
        nc.vector.tensor_mul(rr[:], r0[:], r0[:])
        nc.vector.tensor_mul(rr[:], rr[:], h_sb[:])
        nc.vector.scalar_tensor_tensor(out=rs[:], in0=rr[:], scalar=1.5,
                                       in1=r0[:], op0=SUB, op1=MULT)
        # broadcast -rs (2,64) -> (128,64): bc[p,n] = -rs[p//64, n]
        bc_ps = ps_sm.tile([128, 512], f32, tag="sm", name=f"bc{it}")
        nc.tensor.matmul(bc_ps[:, 0:64], o2t[:], rs[:])

        if last:
            v32 = small.tile([128, 64], f32, tag="v32")
            nc.vector.tensor_mul(v32[:], t_sb[:], bc_ps[:, 0:64])
            # raw dump; host reorders (out[b,2oc+e,d] = v32[e*64+d, b*8+oc])
            nc.sync.dma_start(out=out_d, in_=v32[:])
            break

        # ---------- scatter v = t*bc into block-diag vblk (fused) -------
        # vblk[e*64+d, oc, b*16+2oc+e] = t_sb[e*64+d, b*8+oc] * bc[...]
        pitch = vblk[:].ap[0][0]
        for e in range(2):
            dst = bass.AP(tensor=vblk[:].tensor,
                          offset=64 * e * pitch + e,
                          ap=[[pitch, 64], [130, OC], [16, BL]])
            nc.vector.tensor_tensor(
                out=dst,
                in0=t_sb[64 * e:64 * e + 64, :].rearrange(
                    "p (b o) -> p o b", o=OC),
                in1=bc_ps[64 * e:64 * e + 64, 0:64].rearrange(
                    "p (b o) -> p o b", o=OC),
                op=MULT)

        # ---------- P^T = Vblk^T W^T, h-halves pipelined ----------
        pT_ps = ps_big.tile([128, 1024], f32, tag="big", name=f"pT{it}")
        _warm(pT_ps, 3 if it == 0 else 8, big=(it > 0))
        pT_sb = work.tile([128, 1024], f16, tag="pT_sb")
        p_sb = work.tile([128, HC, 128], f16, tag="p_sb")
        for half in range(2):
            hs = slice(512 * half, 512 * half + 512)
            for oc in range(OC):
                nc.tensor.matmul(
                    pT_ps[:, hs],
                    vblk[:, oc, :],
                    wt16[:, oc, hs],
                    start=(oc == 0), stop=(oc == OC - 1),
                    skip_group_check=True)
            _act_copy(nc, pT_sb[:, hs], pT_ps[:, hs])
            _tq(it).dma_start_transpose(p_sb[:, 4 * half:4 * half + 4, :],
                                        pT_sb[:, hs])

        # ---------- upd^T = P^T X (col-tiled per batch) ----------
        # 2 groups of 4 batches (4-way col-tiled); each group's softmax
        # tail chain hides under the other group's matmuls / next y-g0.
        u_ps = [ps_u.tile([128, 512], f32, tag="u", name=f"u{it}_{g}")
                for g in range(2)]
        u_sb = work.tile([128, 2, 512], f16, tag="u_sb")
        ut = work.tile([128, 2, SC, 128], f16, tag="ut")
        _warm(u_ps[0], 3 if it == 0 else 6, big=(it > 0))
        for g in range(2):
            nc.tensor.matmul(u_ps[g][:], zeros[:], x16[:, 0, 0, 0:512],
                             start=True, stop=False, skip_group_check=True)

        def _upd_tail(g):
            # evac, transpose, logits add, softmax -> c16 for group g
            _act_copy(nc, u_sb[:, g, :], u_ps[g][:])
            _tq(it).dma_start_transpose(ut[:, g], u_sb[:, g, :])
            srcu = ut[:, g].rearrange("p sc (b_ jp) -> p sc b_ jp", jp=32)
            nc.vector.tensor_add(
                logits[:, :, 4 * g:4 * g + 4, :],
                logits[:, :, 4 * g:4 * g + 4, :], srcu[:, :, :, 0:16])
            ex = small.tile([128, SC, 4, NCAP], f32, tag="ex")
            nc.scalar.activation(out=ex[:],
                                 in_=logits[:, :, 4 * g:4 * g + 4, :],
                                 func=EXP, scale=1.0, alpha=0.0)
            sm = small.tile([128, SC, 4, 1], f32, tag="sm")
            nc.vector.reduce_sum(sm[:], ex[:], axis=mybir.AxisListType.X)
            rc = small.tile([128, SC, 4, 1], f32, tag="rc")
            nc.vector.reciprocal(rc[:], sm[:])
            nc.vector.tensor_mul(c16[:, :, 4 * g:4 * g + 4, 0:16], ex[:],
                                 rc.broadcast_to([128, SC, 4, NCAP]))

        # hcq-outer so the p_sb-h1 transpose wait hides under g1's
        # hcq0 matmuls; tail-g0 between the hcq1 blocks so it hides
        # under g1-hcq1; tail-g1 hides under the next y-g0.
        for hcq in range(2):
            for g in range(2):
                for hc in range(4 * hcq, 4 * hcq + 4):
                    for b_ in range(4):
                        b = 4 * g + b_
                        nc.tensor.matmul(
                            u_ps[g][32 * b_:32 * b_ + 16, :],
                            p_sb[:, hc, 16 * b:16 * b + 16],
                            xt16[:, b, hc, :],
                            start=False, stop=(hc == 7),
                            skip_group_check=True,
                            tile_position=(0, 32 * b_))
                if hcq == 1:
                    _upd_tail(g)
    ctx.close()


_CACHE = {}


def _host_consts():
    c0 = np.zeros((128, SC, BL, 32), np.float16)
    c0[:, :, :, 0:NCAP] = np.float16(1.0 / NCAP)
    logi = np.zeros((128, SC, BL, NCAP), np.float32)
    vblk0 = np.zeros((128, OC, 128), np.float16)
    ones2 = np.zeros((128, 2), np.float16)
    ones2[0:64, 0] = 1.0
    ones2[64:128, 1] = 1.0
    o2t = np.zeros((2, 128), np.float32)
    o2t[0, 0:64] = -1.0          # sign-fix for the single-Newton rsqrt
    o2t[1, 64:128] = -1.0
    zeros = np.zeros((128, 128), np.float16)
    magic = np.full((2, 1), MAGIC, np.int32)
    return {"c0i": c0, "logi": logi, "vblki": vblk0, "ones2": ones2,
            "o2t": o2t, "zeros": zeros, "magic": magic}


def _get_nc():
    if "nc" not in _CACHE:
        nc = bacc.Bacc("TRN2", target_bir_lowering=False, debug=False)
        x_d = nc.dram_tensor("x16", [128, BL, SC, 1024], f16,
                             kind="ExternalInput")
        xt_d = nc.dram_tensor("xt16", [128, BL, HC, 512], f16,
                              kind="ExternalInput")
        w_d = nc.dram_tensor("w16", [128, HC, 1024], f16,
                             kind="ExternalInput")
        wt_d = nc.dram_tensor("wt16", [128, OC, 1024], f16,
                              kind="ExternalInput")
        c0_d = nc.dram_tensor("c0i", [128, SC, BL, 32], f16,
                              kind="ExternalInput")
        logits_d = nc.dram_tensor("logi", [128, SC, BL, NCAP], f32,
                                  kind="ExternalInput")
        vblk_d = nc.dram_tensor("vblki", [128, OC, 128], f16,
                                kind="ExternalInput")
        ones2_d = nc.dram_tensor("ones2", [128, 2], f16, kind="ExternalInput")
        o2t_d = nc.dram_tensor("o2t", [2, 128], f32, kind="ExternalInput")
        zeros_d = nc.dram_tensor("zeros", [128, 128], f16,
                                 kind="ExternalInput")
        magic_d = nc.dram_tensor("magic", [2, 1], i32, kind="ExternalInput")
        out_d = nc.dram_tensor("out", [128, 64], f32, kind="ExternalOutput")
        with tile.TileContext(nc) as tc:
            _build_kernel(tc, out_d.ap(), x_d.ap(), xt_d.ap(), w_d.ap(),
                          wt_d.ap(), c0_d.ap(), logits_d.ap(), vblk_d.ap(),
                          ones2_d.ap(), o2t_d.ap(), zeros_d.ap(),
                          magic_d.ap())
        nc.compile()
        _CACHE["nc"] = nc
    return _CACHE["nc"]


def kernel(inputs: np.ndarray, W: np.ndarray, _trace: bool = False):
    """inputs: (512, 64, 1024) f32; W: (1, 1024, 1024) f32.
    Returns (64, 16, 64) f32."""
    nc = _get_nc()
    consts = _host_consts()
    w0 = W[0].astype(np.float16)
    w16h = np.ascontiguousarray(
        w0.reshape(HC, 128, 1024).transpose(1, 0, 2))
    wt16h = np.ascontiguousarray(
        w0.reshape(1024, OC, 128).transpose(2, 1, 0))
    x16f = inputs.astype(np.float16)              # (S, B, H)
    in_maps = []
    for c in range(N_CORES):
        xs = x16f[:, c * BL:(c + 1) * BL, :]      # (S, BL, H)
        x16h = np.ascontiguousarray(
            xs.reshape(SC, 128, BL, H).transpose(1, 2, 0, 3))
        xt16h = np.ascontiguousarray(
            xs.reshape(S, BL, HC, 128).transpose(3, 1, 2, 0))
        m = {"x16": x16h, "xt16": xt16h, "w16": w16h, "wt16": wt16h}
        m.update(consts)
        in_maps.append(m)
    kw = {}
    if _trace:
        kw = dict(trace=True, trace_cores=[0], stitch_traces=False)
    res = run_bass_kernel_spmd(nc, in_maps, core_ids=list(range(N_CORES)),
                               **kw)
    outs = []
    for c in range(N_CORES):
        v = res.results[c]["out"]          # (128=e*64+d, 64=b*8+oc)
        outs.append(v.reshape(2, 64, BL, 8).transpose(2, 3, 0, 1)
                     .reshape(BL, NCAP, DCAP))
    out = np.concatenate(outs, axis=0)
    if _trace:
        return out.astype(np.float32), res
    return out.astype(np.float32)


# revision 24
# speedup vs baseline: 1.2652x; 1.0836x over previous
"""Trainium2 Bass kernel for capsule dynamic routing (nn_Capsule) — v4.

Reference (per batch item b):
    u = x_b @ W; logits = 0
    for i in 4:
        c = softmax(logits, axis=capsule)
        t_j = sum_s c[s,j] * u[s, j*64:(j+1)*64]; v = squash(t)
        if i < 3: logits[s,j] += u[s, jblk] . v_j

Never materializes u (linearity):
    y_j   = sum_s c[s,j] x_s            y-GEMM   (c stationary, col-tiled)
    t     = W^T y^T                     t-GEMM   (w16 stationary per-slice)
    P^T   = Vblk^T W^T                  P-GEMM   (vblk stationary, block-diag)
    upd^T = P^T X                       upd-GEMM (P slices stationary, col-tiled)

v4 vs v3 (230us): trace showed PE active only 53%, HAM cold 40% of span.
  - queue discipline: ALL bulk input loads ride the scalar HWDGE queue;
    the sync queue carries only tiny consts then mid-iteration
    transposes.  (v3 split loads across both queues, so iteration-0/1
    transposes sat 40+us behind xt16 loads -> PE starved -> HAM cold.)
  - load order w16 -> x16 (per b,half chunks) -> wt16 -> xt16 (per
    b,hcq chunks), matching first-use order; iteration-0 y-GEMM is
    emitted batch-major so it consumes x16 chunks as they land.
  - per-stage emission reordered so every stage-bridging evac/
    transpose/softmax hides under another block's matmuls:
      y: half-outer (iters>=1) with per-(half,g) evac+transpose;
      upd: per-g softmax tail emitted between the two hcq1 g-blocks;
      t: per-oc block-diag extract emitted right after that oc's last
      accumulation matmul.
  - transposes alternate sync/scalar queues once loads are done.
  - warm fillers only where a real >3us PE gap is unavoidable
    (pre-stage), rhs=zeros so they never dep on input DMAs.

HW lessons kept from v3:
  - DVE copy PSUM(f32)->SBUF(f16) kills the device; PSUM->f16 casts go
    through ScalarE activation(Copy).
  - matmul start=True lazily zeroes the whole 2KB PSUM bank: accumulation
    groups must own a (partition-range x bank) region exclusively;
    partition-disjoint groups interleave with skip_group_check=True;
    column-disjoint writes into one bank are fine after the first
    start=True (has_written is per-element).
  - squash rsqrt on DVE (bitcast seed + 2 Newton steps); ScalarE runs
    only Copy+Exp -> exactly ONE ACT table load for the whole kernel.
  - nc.vector.memset on f16 tiles is unreliable: constants come from host.
"""
import numpy as np
from contextlib import ExitStack

import concourse.bass as bass
import concourse.bacc as bacc
import concourse.tile as tile
from concourse import mybir
from concourse.bass_utils import run_bass_kernel_spmd

f16 = mybir.dt.float16
f32 = mybir.dt.float32
i32 = mybir.dt.int32
COPY = mybir.ActivationFunctionType.Copy
EXP = mybir.ActivationFunctionType.Exp
MULT = mybir.AluOpType.mult
SUB = mybir.AluOpType.subtract
ADD = mybir.AluOpType.add
SHR = mybir.AluOpType.logical_shift_right

S, B, H = 512, 64, 1024
NCAP, DCAP = 16, 64
ROUTINGS = 4
N_CORES = 8
BL = B // N_CORES          # 8 batch items per core
SC = S // 128              # 4 s-chunks
HC = H // 128              # 8 h-chunks
OC = H // 128              # 8 o-chunks (o = NCAP*DCAP = 1024)
MAGIC = 0x5EF759DF         # rsqrt seed for h = s/2: 0x5f3759df - (1<<22)


def _act_copy(nc, out, in_):
    nc.scalar.activation(out=out, in_=in_, func=COPY, scale=1.0, alpha=0.0)


def _build_kernel(tc, out_d, x_d, xt_d, w_d, wt_d, c0_d, logits_d, vblk_d,
                  ones2_d, o2t_d, zeros_d, magic_d, ident_d):
    nc = tc.nc
    ctx = ExitStack()
    const = ctx.enter_context(tc.tile_pool(name="const", bufs=1))
    work = ctx.enter_context(tc.tile_pool(name="work", bufs=1))
    small = ctx.enter_context(tc.tile_pool(name="small", bufs=2))
    ps_big = ctx.enter_context(tc.tile_pool(name="ps_big", bufs=2,
                                            space="PSUM"))
    ps_u = ctx.enter_context(tc.tile_pool(name="ps_u", bufs=2, space="PSUM"))
    ps_sm = ctx.enter_context(tc.tile_pool(name="ps_sm", bufs=2,
                                           space="PSUM"))

    # ---------- persistent tensors ----------
    x16 = const.tile([128, BL, SC, 1024], f16)      # (s_loc, b, sc, h)
    xt16 = const.tile([128, BL, HC, 512], f16)      # (h_loc, b, hc, s)
    w16 = const.tile([128, HC, 1024], f16)          # (h_loc, hc, o)
    wt16 = const.tile([128, OC, 1024], f16)         # (o_loc, oc, h)
    c16 = const.tile([128, SC, BL, 32], f16)        # coeffs, cols 16-31 zero
    logits = const.tile([128, SC, BL, NCAP], f32)
    vblk = const.tile([128, OC, 128], f16)          # block-diag v, bj dense
    ones2 = const.tile([128, 2], f16)               # [[1;0],[0;1]] halves
    o2t = const.tile([2, 128], f32)                 # broadcast helper
    zeros = const.tile([128, 128], f16)             # zero-weight psum fill
    magic = const.tile([2, 1], i32)                 # rsqrt seed constant
    ident = const.tile([128, 128], f16)             # PE-transpose identity

    # ---------- loads ----------
    # ALL DMAs (loads + transposes) ride the sync (SP) queue: HWDGE DMA
    # issue blocks the issuing engine for ~0.5-1.5us per call, so the
    # scalar (ACT) engine must never issue DMAs or its evacs stall.
    # Loads are big per-b contiguous chunks in first-use order.
    nc.sync.dma_start(out=zeros[:], in_=zeros_d[:])
    nc.sync.dma_start(out=ones2[:], in_=ones2_d[:])
    nc.sync.dma_start(out=o2t[:], in_=o2t_d[:])
    nc.sync.dma_start(out=magic[:], in_=magic_d[:])
    nc.sync.dma_start(out=ident[:], in_=ident_d[:])
    nc.sync.dma_start(out=c16[:], in_=c0_d[:])
    nc.sync.dma_start(out=logits[:], in_=logits_d[:])
    nc.sync.dma_start(out=vblk[:], in_=vblk_d[:])
    # x16 b0-3 first so iteration-0 y-GEMM starts ASAP; w16 before x b4-7
    # (t-GEMM needs it right after y); wt16 before xt16 (P before upd).
    for b in range(4):
        nc.sync.dma_start(out=x16[:, b], in_=x_d[:, b])
    nc.sync.dma_start(out=w16[:], in_=w_d[:])
    for b in range(4, BL):
        nc.sync.dma_start(out=x16[:, b], in_=x_d[:, b])
    nc.sync.dma_start(out=wt16[:], in_=wt_d[:])
    for b in range(BL):
        nc.sync.dma_start(out=xt16[:, b], in_=xt_d[:, b])

    def _warm(ps_tile, n, big=False):
        # zero-weight matmuls into a psum tile whose next real producer
        # begins with start=True (which wipes the bank): pure HAM fuel
        # that runs during the dependency-wait gap before the stage.
        # big=True streams x16 (N=512, 215ns each) for post-load fillers;
        # otherwise rhs=zeros so fillers never wait on input DMAs.
        for k in range(n):
            if big:
                nc.tensor.matmul(ps_tile[:, 0:512], zeros[:],
                                 x16[:, 0, 0, 0:512],
                                 start=(k == 0), stop=False,
                                 skip_group_check=True)
            else:
                nc.tensor.matmul(ps_tile[:, 0:128], zeros[:], zeros[:],
                                 start=(k == 0), stop=False,
                                 skip_group_check=True)

    def _tr(it, out3d, in2d, name):
        # SBUF transpose.  Iteration 0 uses PE transpose-mode (the HWDGE
        # transpose barrier + shared sem pool would serialize DMA
        # transposes against the in-flight input loads, stalling both);
        # PE is idle during the load window anyway and this keeps HAM
        # warm.  Iterations >=1 use xbar DMA transposes on sync.
        if it == 0:
            n = in2d.free_size() // 128
            tp = ps_sm.tile([128, 512], f16, tag="sm", name=name)
            for k in range(n):
                nc.tensor.matmul(tp[:, 128 * k:128 * k + 128],
                                 in2d[:, 128 * k:128 * k + 128], ident[:],
                                 is_transpose=True, start=(k == 0),
                                 stop=(k == n - 1), skip_group_check=True)
            _act_copy(nc, out3d,
                      tp[:, 0:128 * n].rearrange("p (a b) -> p a b", b=128))
        else:
            nc.sync.dma_start_transpose(out3d, in2d)

    for it in range(ROUTINGS):
        last = it == ROUTINGS - 1

        # ---------- y = C^T X ----------
        y_ps = [ps_big.tile([128, 1024], f32, tag="big", name=f"y{it}_{g}")
                for g in range(2)]
        if it == 0:
            _warm(y_ps[0], 16)
            import os
            if os.environ.get("DIAG_FILL"):
                _warm(y_ps[0], 300, big=True)
        else:
            # cover the tail-g1 softmax chain + keep HAM warm
            _warm(y_ps[0], 8, big=True)
        y_sb = work.tile([128, 2, 1024], f16, tag="y_sb")
        yt = work.tile([128, HC, 256], f16, tag="yt")

        def _y_mm(g, b_, half, sc):
            b = 4 * g + b_
            hs = slice(512 * half, 512 * half + 512)
            nc.tensor.matmul(
                y_ps[g][32 * b_:32 * b_ + 32, hs],
                c16[:, sc, b, :],
                x16[:, b, sc, hs],
                start=(sc == 0), stop=(sc == SC - 1),
                skip_group_check=True,
                tile_position=(0, 32 * b_))

        def _y_evac(g, half):
            hs = slice(512 * half, 512 * half + 512)
            _act_copy(nc, y_sb[:, g, hs], y_ps[g][:, hs])
            _tr(it, yt[:, 4 * half:4 * half + 4, 128 * g:128 * g + 128],
                y_sb[:, g, hs], f"ytr{it}_{half}_{g}")

        if it == 0:
            # batch-major: track x16 arrival order b0..b7
            for g in range(2):
                for b_ in range(4):
                    for half in range(2):
                        for sc in range(SC):
                            _y_mm(g, b_, half, sc)
                for half in range(2):
                    _y_evac(g, half)
        else:
            # g-outer: y-g0 runs while upd tails s2/s3 finish; y-g1's
            # c16 is ready by the time y-g0's matmuls complete
            for g in range(2):
                for half in range(2):
                    for b_ in range(4):
                        for sc in range(SC):
                            _y_mm(g, b_, half, sc)
                    _y_evac(g, half)

        # ---------- t = W^T y^T, two hc passes ----------
        yt_dense = yt.rearrange("p hc (g b_ jp) -> p hc g b_ jp", g=2, jp=32)
        t_ps = ps_big.tile([128, 1024], f32, tag="big", name=f"t{it}")
        _warm(t_ps, 2 if it == 0 else 4, big=(it > 0))
        for hcq in range(2):
            for oc in range(OC):
                for hc in range(4 * hcq, 4 * hcq + 4):
                    nc.tensor.matmul(
                        t_ps[:, oc * 128:oc * 128 + 128],
                        w16[:, hc, oc * 128:oc * 128 + 128],
                        yt_dense[:, hc, :, :, 0:16],
                        start=(hcq == 0 and hc == 0 and oc % 4 == 0),
                        stop=(hcq == 1 and hc == 7),
                        skip_group_check=True)
            if hcq == 0 and it > 0:
                # bridge the yt-h1 transpose wait before the hcq1 pass
                fb = ps_sm.tile([128, 512], f32, tag="sm", name=f"tf{it}")
                _warm(fb, 5, big=True)

        # ---------- extract block-diag: t_sb (e*64+d, b*8+oc) ----------
        # t_ps col for (oc, b, j=2oc+e) = oc*130 + b*16 + e: linear in
        # (oc, b), so one strided copy per e-half (DVE + ScalarE in
        # parallel) replaces 16 tiny per-oc copies.
        t_sb = small.tile([128, 64], f32, tag="t_sb")
        pps = t_ps[:].ap[0][0]
        psb = t_sb[:].ap[0][0]
        for e in range(2):
            srcd = bass.AP(tensor=t_ps[:].tensor, offset=64 * e * pps + e,
                           ap=[[pps, 64], [130, OC], [16, BL]])
            dstd = bass.AP(tensor=t_sb[:].tensor, offset=64 * e * psb,
                           ap=[[psb, 64], [1, OC], [8, BL]])
            if e == 0:
                nc.vector.tensor_copy(dstd, srcd)
            else:
                _act_copy(nc, dstd, srcd)

        # ---------- squash: rs = rsqrt(sum_d t^2 + eps) on DVE ----------
        t2 = small.tile([128, 64], f16, tag="t2")
        nc.vector.tensor_mul(t2[:], t_sb[:], t_sb[:])
        sq_ps = ps_sm.tile([2, 512], f32, tag="sm", name=f"sq{it}")
        nc.tensor.matmul(sq_ps[:, 0:64], ones2[:], t2[:])
        h_sb = small.tile([2, 64], f32, tag="h_sb")
        nc.vector.tensor_scalar(out=h_sb[:], in0=sq_ps[:, 0:64],
                                scalar1=0.5, scalar2=5e-8, op0=MULT, op1=ADD)
        ri = small.tile([2, 64], i32, tag="ri")
        nc.vector.tensor_scalar(out=ri[:], in0=h_sb.bitcast(i32),
                                scalar1=1, scalar2=None, op0=SHR)
        r0 = small.tile([2, 64], f32, tag="r0")
        nc.vector.tensor_tensor(out=r0.bitcast(i32),
                                in0=magic.broadcast_to([2, 64]),
                                in1=ri[:], op=SUB)
        # Newton x1 with sign fold: rs = (h*r0*r0 - 1.5) * r0 = -rsqrt
        # approx (0.2% err, fine at 2e-2 tol); o2t carries -1 so the
        # broadcast flips the sign back.
        rr = small.tile([2, 64], f32, tag="rr")
        rs = small.tile([2, 64], f32, tag="rs")
        nc.vector.tensor_mul(rr[:], r0[:], r0[:])
        nc.vector.tensor_mul(rr[:], rr[:], h_sb[:])
        nc.vector.scalar_tensor_tensor(out=rs[:], in0=rr[:], scalar=1.5,
                                       in1=r0[:], op0=SUB, op1=MULT)
        # broadcast -rs (2,64) -> (128,64): bc[p,n] = -rs[p//64, n]
        bc_ps = ps_sm.tile([128, 512], f32, tag="sm", name=f"bc{it}")
        nc.tensor.matmul(bc_ps[:, 0:64], o2t[:], rs[:])

        if last:
            v32 = small.tile([128, 64], f32, tag="v32")
            nc.vector.tensor_mul(v32[:], t_sb[:], bc_ps[:, 0:64])
            # raw dump; host reorders (out[b,2oc+e,d] = v32[e*64+d, b*8+oc])
            nc.sync.dma_start(out=out_d, in_=v32[:])
            break

        # ---------- scatter v = t*bc into block-diag vblk (fused) -------
        # vblk[e*64+d, oc, b*16+2oc+e] = t_sb[e*64+d, b*8+oc] * bc[...]
        pitch = vblk[:].ap[0][0]
        for e in range(2):
            dst = bass.AP(tensor=vblk[:].tensor,
                          offset=64 * e * pitch + e,
                          ap=[[pitch, 64], [130, OC], [16, BL]])
            nc.vector.tensor_tensor(
                out=dst,
                in0=t_sb[64 * e:64 * e + 64, :].rearrange(
                    "p (b o) -> p o b", o=OC),
                in1=bc_ps[64 * e:64 * e + 64, 0:64].rearrange(
                    "p (b o) -> p o b", o=OC),
                op=MULT)

        # ---------- P^T = Vblk^T W^T, h-halves pipelined ----------
        pT_ps = ps_big.tile([128, 1024], f32, tag="big", name=f"pT{it}")
        _warm(pT_ps, 3 if it == 0 else 8, big=(it > 0))
        pT_sb = work.tile([128, 1024], f16, tag="pT_sb")
        p_sb = work.tile([128, HC, 128], f16, tag="p_sb")
        for half in range(2):
            hs = slice(512 * half, 512 * half + 512)
            for oc in range(OC):
                nc.tensor.matmul(
                    pT_ps[:, hs],
                    vblk[:, oc, :],
                    wt16[:, oc, hs],
                    start=(oc == 0), stop=(oc == OC - 1),
                    skip_group_check=True)
            _act_copy(nc, pT_sb[:, hs], pT_ps[:, hs])
            _tr(it, p_sb[:, 4 * half:4 * half + 4, :], pT_sb[:, hs],
                f"ptr{it}_{half}")

        # ---------- upd^T = P^T X (col-tiled per batch) ----------
        # 2 groups of 4 batches (4-way col-tiled); each group's softmax
        # tail chain hides under the other group's matmuls / next y-g0.
        u_ps = [ps_u.tile([128, 512], f32, tag="u", name=f"u{it}_{g}")
                for g in range(2)]
        u_sb = work.tile([128, 2, 512], f16, tag="u_sb")
        ut = work.tile([128, 2, SC, 128], f16, tag="ut")
        _warm(u_ps[0], 3 if it == 0 else 6, big=(it > 0))
        for g in range(2):
            nc.tensor.matmul(u_ps[g][:], zeros[:], x16[:, 0, 0, 0:512],
                             start=True, stop=False, skip_group_check=True)

        def _upd_tail(g):
            # evac, transpose, logits add, softmax -> c16 for group g
            _act_copy(nc, u_sb[:, g, :], u_ps[g][:])
            _tr(it, ut[:, g], u_sb[:, g, :], f"utr{it}_{g}")
            srcu = ut[:, g].rearrange("p sc (b_ jp) -> p sc b_ jp", jp=32)
            nc.vector.tensor_add(
                logits[:, :, 4 * g:4 * g + 4, :],
                logits[:, :, 4 * g:4 * g + 4, :], srcu[:, :, :, 0:16])
            ex = small.tile([128, SC, 4, NCAP], f32, tag="ex")
            nc.scalar.activation(out=ex[:],
                                 in_=logits[:, :, 4 * g:4 * g + 4, :],
                                 func=EXP, scale=1.0, alpha=0.0)
            sm = small.tile([128, SC, 4, 1], f32, tag="sm")
            nc.vector.reduce_sum(sm[:], ex[:], axis=mybir.AxisListType.X)
            rc = small.tile([128, SC, 4, 1], f32, tag="rc")
            nc.vector.reciprocal(rc[:], sm[:])
            nc.vector.tensor_mul(c16[:, :, 4 * g:4 * g + 4, 0:16], ex[:],
                                 rc.broadcast_to([128, SC, 4, NCAP]))

        # hcq-outer so the p_sb-h1 transpose wait hides under g1's
        # hcq0 matmuls; tail-g0 between the hcq1 blocks so it hides
        # under g1-hcq1; tail-g1 hides under the next y-g0.
        for hcq in range(2):
            for g in range(2):
                for hc in range(4 * hcq, 4 * hcq + 4):
                    for b_ in range(4):
                        b = 4 * g + b_
                        nc.tensor.matmul(
                            u_ps[g][32 * b_:32 * b_ + 16, :],
                            p_sb[:, hc, 16 * b:16 * b + 16],
                            xt16[:, b, hc, :],
                            start=False, stop=(hc == 7),
                            skip_group_check=True,
                            tile_position=(0, 32 * b_))
                if hcq == 1:
                    _upd_tail(g)
    ctx.close()


_CACHE = {}


def _host_consts():
    c0 = np.zeros((128, SC, BL, 32), np.float16)
    c0[:, :, :, 0:NCAP] = np.float16(1.0 / NCAP)
    logi = np.zeros((128, SC, BL, NCAP), np.float32)
    vblk0 = np.zeros((128, OC, 128), np.float16)
    ones2 = np.zeros((128, 2), np.float16)
    ones2[0:64, 0] = 1.0
    ones2[64:128, 1] = 1.0
    o2t = np.zeros((2, 128), np.float32)
    o2t[0, 0:64] = -1.0          # sign-fix for the single-Newton rsqrt
    o2t[1, 64:128] = -1.0
    zeros = np.zeros((128, 128), np.float16)
    magic = np.full((2, 1), MAGIC, np.int32)
    ident = np.eye(128, dtype=np.float16)
    return {"c0i": c0, "logi": logi, "vblki": vblk0, "ones2": ones2,
            "o2t": o2t, "zeros": zeros, "magic": magic, "ident": ident}


def _get_nc():
    if "nc" not in _CACHE:
        nc = bacc.Bacc("TRN2", target_bir_lowering=False, debug=False)
        x_d = nc.dram_tensor("x16", [128, BL, SC, 1024], f16,
                             kind="ExternalInput")
        xt_d = nc.dram_tensor("xt16", [128, BL, HC, 512], f16,
                              kind="ExternalInput")
        w_d = nc.dram_tensor("w16", [128, HC, 1024], f16,
                             kind="ExternalInput")
        wt_d = nc.dram_tensor("wt16", [128, OC, 1024], f16,
                              kind="ExternalInput")
        c0_d = nc.dram_tensor("c0i", [128, SC, BL, 32], f16,
                              kind="ExternalInput")
        logits_d = nc.dram_tensor("logi", [128, SC, BL, NCAP], f32,
                                  kind="ExternalInput")
        vblk_d = nc.dram_tensor("vblki", [128, OC, 128], f16,
                                kind="ExternalInput")
        ones2_d = nc.dram_tensor("ones2", [128, 2], f16, kind="ExternalInput")
        o2t_d = nc.dram_tensor("o2t", [2, 128], f32, kind="ExternalInput")
        zeros_d = nc.dram_tensor("zeros", [128, 128], f16,
                                 kind="ExternalInput")
        magic_d = nc.dram_tensor("magic", [2, 1], i32, kind="ExternalInput")
        ident_d = nc.dram_tensor("ident", [128, 128], f16,
                                 kind="ExternalInput")
        out_d = nc.dram_tensor("out", [128, 64], f32, kind="ExternalOutput")
        with tile.TileContext(nc) as tc:
            _build_kernel(tc, out_d.ap(), x_d.ap(), xt_d.ap(), w_d.ap(),
                          wt_d.ap(), c0_d.ap(), logits_d.ap(), vblk_d.ap(),
                          ones2_d.ap(), o2t_d.ap(), zeros_d.ap(),
                          magic_d.ap(), ident_d.ap())
        nc.compile()
        _CACHE["nc"] = nc
    return _CACHE["nc"]


def kernel(inputs: np.ndarray, W: np.ndarray, _trace: bool = False):
    """inputs: (512, 64, 1024) f32; W: (1, 1024, 1024) f32.
    Returns (64, 16, 64) f32."""
    nc = _get_nc()
    consts = _host_consts()
    w0 = W[0].astype(np.float16)
    w16h = np.ascontiguousarray(
        w0.reshape(HC, 128, 1024).transpose(1, 0, 2))
    wt16h = np.ascontiguousarray(
        w0.reshape(1024, OC, 128).transpose(2, 1, 0))
    x16f = inputs.astype(np.float16)              # (S, B, H)
    in_maps = []
    for c in range(N_CORES):
        xs = x16f[:, c * BL:(c + 1) * BL, :]      # (S, BL, H)
        x16h = np.ascontiguousarray(
            xs.reshape(SC, 128, BL, H).transpose(1, 2, 0, 3))
        xt16h = np.ascontiguousarray(
            xs.reshape(S, BL, HC, 128).transpose(3, 1, 2, 0))
        m = {"x16": x16h, "xt16": xt16h, "w16": w16h, "wt16": wt16h}
        m.update(consts)
        in_maps.append(m)
    kw = {}
    if _trace:
        kw = dict(trace=True, trace_cores=[0], stitch_traces=False)
    res = run_bass_kernel_spmd(nc, in_maps, core_ids=list(range(N_CORES)),
                               **kw)
    outs = []
    for c in range(N_CORES):
        v = res.results[c]["out"]          # (128=e*64+d, 64=b*8+oc)
        outs.append(v.reshape(2, 64, BL, 8).transpose(2, 3, 0, 1)
                     .reshape(BL, NCAP, DCAP))
    out = np.concatenate(outs, axis=0)
    if _trace:
        return out.astype(np.float32), res
    return out.astype(np.float32)


# revision 26
# speedup vs baseline: 1.3502x; 1.0672x over previous
"""Trainium2 Bass kernel for capsule dynamic routing (nn_Capsule) — v4.

Reference (per batch item b):
    u = x_b @ W; logits = 0
    for i in 4:
        c = softmax(logits, axis=capsule)
        t_j = sum_s c[s,j] * u[s, j*64:(j+1)*64]; v = squash(t)
        if i < 3: logits[s,j] += u[s, jblk] . v_j

Never materializes u (linearity):
    y_j   = sum_s c[s,j] x_s            y-GEMM   (c stationary, col-tiled)
    t     = W^T y^T                     t-GEMM   (w16 stationary per-slice)
    P^T   = Vblk^T W^T                  P-GEMM   (vblk stationary, block-diag)
    upd^T = P^T X                       upd-GEMM (P slices stationary, col-tiled)

v4 vs v3 (230us): trace showed PE active only 53%, HAM cold 40% of span.
  - queue discipline: ALL bulk input loads ride the scalar HWDGE queue;
    the sync queue carries only tiny consts then mid-iteration
    transposes.  (v3 split loads across both queues, so iteration-0/1
    transposes sat 40+us behind xt16 loads -> PE starved -> HAM cold.)
  - load order w16 -> x16 (per b,half chunks) -> wt16 -> xt16 (per
    b,hcq chunks), matching first-use order; iteration-0 y-GEMM is
    emitted batch-major so it consumes x16 chunks as they land.
  - per-stage emission reordered so every stage-bridging evac/
    transpose/softmax hides under another block's matmuls:
      y: half-outer (iters>=1) with per-(half,g) evac+transpose;
      upd: per-g softmax tail emitted between the two hcq1 g-blocks;
      t: per-oc block-diag extract emitted right after that oc's last
      accumulation matmul.
  - transposes alternate sync/scalar queues once loads are done.
  - warm fillers only where a real >3us PE gap is unavoidable
    (pre-stage), rhs=zeros so they never dep on input DMAs.

HW lessons kept from v3:
  - DVE copy PSUM(f32)->SBUF(f16) kills the device; PSUM->f16 casts go
    through ScalarE activation(Copy).
  - matmul start=True lazily zeroes the whole 2KB PSUM bank: accumulation
    groups must own a (partition-range x bank) region exclusively;
    partition-disjoint groups interleave with skip_group_check=True;
    column-disjoint writes into one bank are fine after the first
    start=True (has_written is per-element).
  - squash rsqrt on DVE (bitcast seed + 2 Newton steps); ScalarE runs
    only Copy+Exp -> exactly ONE ACT table load for the whole kernel.
  - nc.vector.memset on f16 tiles is unreliable: constants come from host.
"""
import numpy as np
from contextlib import ExitStack

import concourse.bass as bass
import concourse.bacc as bacc
import concourse.tile as tile
from concourse import mybir
from concourse.bass_utils import run_bass_kernel_spmd

f16 = mybir.dt.float16
f32 = mybir.dt.float32
i32 = mybir.dt.int32
COPY = mybir.ActivationFunctionType.Copy
EXP = mybir.ActivationFunctionType.Exp
MULT = mybir.AluOpType.mult
SUB = mybir.AluOpType.subtract
ADD = mybir.AluOpType.add
SHR = mybir.AluOpType.logical_shift_right

S, B, H = 512, 64, 1024
NCAP, DCAP = 16, 64
ROUTINGS = 4
N_CORES = 8
BL = B // N_CORES          # 8 batch items per core
SC = S // 128              # 4 s-chunks
HC = H // 128              # 8 h-chunks
OC = H // 128              # 8 o-chunks (o = NCAP*DCAP = 1024)
MAGIC = 0x5EF759DF         # rsqrt seed for h = s/2: 0x5f3759df - (1<<22)


def _act_copy(nc, out, in_):
    nc.scalar.activation(out=out, in_=in_, func=COPY, scale=1.0, alpha=0.0)


def _build_kernel(tc, out_d, x_d, xt_d, w_d, wt_d, c0_d, logits_d, vblk_d,
                  ones2_d, o2t_d, zeros_d, magic_d, ident_d):
    nc = tc.nc
    ctx = ExitStack()
    const = ctx.enter_context(tc.tile_pool(name="const", bufs=1))
    work = ctx.enter_context(tc.tile_pool(name="work", bufs=1))
    small = ctx.enter_context(tc.tile_pool(name="small", bufs=2))
    ps_big = ctx.enter_context(tc.tile_pool(name="ps_big", bufs=2,
                                            space="PSUM"))
    ps_u = ctx.enter_context(tc.tile_pool(name="ps_u", bufs=2, space="PSUM"))
    ps_sm = ctx.enter_context(tc.tile_pool(name="ps_sm", bufs=2,
                                           space="PSUM"))

    # ---------- persistent tensors ----------
    x16 = const.tile([128, BL, SC, 1024], f16)      # (s_loc, b, sc, h)
    xt16 = const.tile([128, BL, HC, 512], f16)      # (h_loc, b, hc, s)
    w16 = const.tile([128, HC, 1024], f16)          # (h_loc, hc, o)
    wt16 = const.tile([128, OC, 1024], f16)         # (o_loc, oc, h)
    c16 = const.tile([128, SC, BL, 32], f16)        # coeffs, cols 16-31 zero
    logits = const.tile([128, SC, BL, NCAP], f32)
    vblk = const.tile([128, OC, 128], f16)          # block-diag v, bj dense
    ones2 = const.tile([128, 2], f16)               # [[1;0],[0;1]] halves
    o2t = const.tile([2, 128], f32)                 # broadcast helper
    zeros = const.tile([128, 128], f16)             # zero-weight psum fill
    magic = const.tile([2, 1], i32)                 # rsqrt seed constant
    ident = const.tile([128, 128], f16)             # PE-transpose identity

    # ---------- loads ----------
    # ALL DMAs (loads + transposes) ride the sync (SP) queue: HWDGE DMA
    # issue blocks the issuing engine for ~0.5-1.5us per call, so the
    # scalar (ACT) engine must never issue DMAs or its evacs stall.
    # Loads are big per-b contiguous chunks in first-use order.
    nc.sync.dma_start(out=zeros[:], in_=zeros_d[:])
    nc.sync.dma_start(out=ones2[:], in_=ones2_d[:])
    nc.sync.dma_start(out=o2t[:], in_=o2t_d[:])
    nc.sync.dma_start(out=magic[:], in_=magic_d[:])
    nc.sync.dma_start(out=ident[:], in_=ident_d[:])
    nc.sync.dma_start(out=c16[:], in_=c0_d[:])
    nc.sync.dma_start(out=logits[:], in_=logits_d[:])
    nc.sync.dma_start(out=vblk[:], in_=vblk_d[:])
    # x16 b0-3 first so iteration-0 y-GEMM starts ASAP; w16 before x b4-7
    # (t-GEMM needs it right after y); wt16 before xt16 (P before upd).
    for b in range(4):
        nc.sync.dma_start(out=x16[:, b], in_=x_d[:, b])
    nc.sync.dma_start(out=w16[:], in_=w_d[:])
    for b in range(4, BL):
        nc.sync.dma_start(out=x16[:, b], in_=x_d[:, b])
    nc.sync.dma_start(out=wt16[:], in_=wt_d[:])
    for b in range(BL):
        nc.sync.dma_start(out=xt16[:, b], in_=xt_d[:, b])

    def _warm(ps_tile, n, big=False):
        # zero-weight matmuls into a psum tile whose next real producer
        # begins with start=True (which wipes the bank): pure HAM fuel
        # that runs during the dependency-wait gap before the stage.
        # big=True streams x16 (N=512, 215ns each) for post-load fillers;
        # otherwise rhs=zeros so fillers never wait on input DMAs.
        for k in range(n):
            if big:
                nc.tensor.matmul(ps_tile[:, 0:512], zeros[:],
                                 x16[:, 0, 0, 0:512],
                                 start=(k == 0), stop=False,
                                 skip_group_check=True)
            else:
                nc.tensor.matmul(ps_tile[:, 0:128], zeros[:], zeros[:],
                                 start=(k == 0), stop=False,
                                 skip_group_check=True)

    def _tr(it, out3d, in2d, name):
        # SBUF transpose.  Iteration 0 uses PE transpose-mode (the HWDGE
        # transpose barrier + shared sem pool would serialize DMA
        # transposes against the in-flight input loads, stalling both);
        # PE is idle during the load window anyway and this keeps HAM
        # warm.  Iterations >=1 use xbar DMA transposes on sync.
        if it == 0:
            n = in2d.free_size() // 128
            tp = ps_sm.tile([128, 512], f16, tag="sm", name=name)
            for k in range(n):
                nc.tensor.matmul(tp[:, 128 * k:128 * k + 128],
                                 in2d[:, 128 * k:128 * k + 128], ident[:],
                                 is_transpose=True, start=(k == 0),
                                 stop=(k == n - 1), skip_group_check=True)
            _act_copy(nc, out3d,
                      tp[:, 0:128 * n].rearrange("p (a b) -> p a b", b=128))
        else:
            nc.sync.dma_start_transpose(out3d, in2d)

    for it in range(ROUTINGS):
        last = it == ROUTINGS - 1

        # ---------- y = C^T X ----------
        y_ps = [ps_big.tile([128, 1024], f32, tag="big", name=f"y{it}_{g}")
                for g in range(2)]
        if it == 0:
            _warm(y_ps[0], 16)
            import os
            if os.environ.get("DIAG_FILL"):
                _warm(y_ps[0], 300, big=True)
        else:
            # cover the tail-g1 softmax chain + keep HAM warm
            _warm(y_ps[0], 8, big=True)
        y_sb = work.tile([128, 2, 1024], f16, tag="y_sb")
        yt = work.tile([128, HC, 256], f16, tag="yt")

        def _y_mm(g, b_, half, sc):
            b = 4 * g + b_
            hs = slice(512 * half, 512 * half + 512)
            nc.tensor.matmul(
                y_ps[g][32 * b_:32 * b_ + 32, hs],
                c16[:, sc, b, :],
                x16[:, b, sc, hs],
                start=(sc == 0), stop=(sc == SC - 1),
                skip_group_check=True,
                tile_position=(0, 32 * b_))

        def _y_evac(g, half):
            hs = slice(512 * half, 512 * half + 512)
            _act_copy(nc, y_sb[:, g, hs], y_ps[g][:, hs])
            _tr(it, yt[:, 4 * half:4 * half + 4, 128 * g:128 * g + 128],
                y_sb[:, g, hs], f"ytr{it}_{half}_{g}")

        if it == 0:
            # batch-major: track x16 arrival order b0..b7
            for g in range(2):
                for b_ in range(4):
                    for half in range(2):
                        for sc in range(SC):
                            _y_mm(g, b_, half, sc)
                for half in range(2):
                    _y_evac(g, half)
        else:
            # g-outer: y-g0 runs while upd tails s2/s3 finish; y-g1's
            # c16 is ready by the time y-g0's matmuls complete
            for g in range(2):
                for half in range(2):
                    for b_ in range(4):
                        for sc in range(SC):
                            _y_mm(g, b_, half, sc)
                    _y_evac(g, half)

        # ---------- t = W^T y^T, two hc passes ----------
        yt_dense = yt.rearrange("p hc (g b_ jp) -> p hc g b_ jp", g=2, jp=32)
        t_ps = ps_big.tile([128, 1024], f32, tag="big", name=f"t{it}")
        _warm(t_ps, 2 if it == 0 else 4, big=(it > 0))
        for hcq in range(2):
            for oc in range(OC):
                for hc in range(4 * hcq, 4 * hcq + 4):
                    nc.tensor.matmul(
                        t_ps[:, oc * 128:oc * 128 + 128],
                        w16[:, hc, oc * 128:oc * 128 + 128],
                        yt_dense[:, hc, :, :, 0:16],
                        start=(hcq == 0 and hc == 0 and oc % 4 == 0),
                        stop=(hcq == 1 and hc == 7),
                        skip_group_check=True)
            if hcq == 0 and it > 0:
                # bridge the yt-h1 transpose wait before the hcq1 pass
                fb = ps_sm.tile([128, 512], f32, tag="sm", name=f"tf{it}")
                _warm(fb, 5, big=True)

        # ---------- extract block-diag: t_sb (e*64+d, b*8+oc) ----------
        # t_ps col for (oc, b, j=2oc+e) = oc*130 + b*16 + e: linear in
        # (oc, b), so one strided copy per e-half (DVE + ScalarE in
        # parallel) replaces 16 tiny per-oc copies.
        t_sb = small.tile([128, 64], f32, tag="t_sb")
        pps = t_ps[:].ap[0][0]
        psb = t_sb[:].ap[0][0]
        for e in range(2):
            srcd = bass.AP(tensor=t_ps[:].tensor, offset=64 * e * pps + e,
                           ap=[[pps, 64], [130, OC], [16, BL]])
            dstd = bass.AP(tensor=t_sb[:].tensor, offset=64 * e * psb,
                           ap=[[psb, 64], [1, OC], [8, BL]])
            if e == 0:
                nc.vector.tensor_copy(dstd, srcd)
            else:
                _act_copy(nc, dstd, srcd)

        # ---------- squash: rs = rsqrt(sum_d t^2 + eps) on DVE ----------
        t2 = small.tile([128, 64], f16, tag="t2")
        nc.vector.tensor_mul(t2[:], t_sb[:], t_sb[:])
        sq_ps = ps_sm.tile([2, 512], f32, tag="sm", name=f"sq{it}")
        nc.tensor.matmul(sq_ps[:, 0:64], ones2[:], t2[:])
        h_sb = small.tile([2, 64], f32, tag="h_sb")
        nc.vector.tensor_scalar(out=h_sb[:], in0=sq_ps[:, 0:64],
                                scalar1=0.5, scalar2=5e-8, op0=MULT, op1=ADD)
        ri = small.tile([2, 64], i32, tag="ri")
        nc.vector.tensor_scalar(out=ri[:], in0=h_sb.bitcast(i32),
                                scalar1=1, scalar2=None, op0=SHR)
        r0 = small.tile([2, 64], f32, tag="r0")
        nc.vector.tensor_tensor(out=r0.bitcast(i32),
                                in0=magic.broadcast_to([2, 64]),
                                in1=ri[:], op=SUB)
        # Newton x1 with sign fold: rs = (h*r0*r0 - 1.5) * r0 = -rsqrt
        # approx (0.2% err, fine at 2e-2 tol); o2t carries -1 so the
        # broadcast flips the sign back.
        rr = small.tile([2, 64], f32, tag="rr")
        rs = small.tile([2, 64], f32, tag="rs")
        nc.vector.tensor_mul(rr[:], r0[:], r0[:])
        nc.vector.tensor_mul(rr[:], rr[:], h_sb[:])
        nc.vector.scalar_tensor_tensor(out=rs[:], in0=rr[:], scalar=1.5,
                                       in1=r0[:], op0=SUB, op1=MULT)
        # broadcast -rs (2,64) -> (128,64): bc[p,n] = -rs[p//64, n]
        bc_ps = ps_sm.tile([128, 512], f32, tag="sm", name=f"bc{it}")
        nc.tensor.matmul(bc_ps[:, 0:64], o2t[:], rs[:])

        if last:
            v32 = small.tile([128, 64], f32, tag="v32")
            nc.vector.tensor_mul(v32[:], t_sb[:], bc_ps[:, 0:64])
            # raw dump; host reorders (out[b,2oc+e,d] = v32[e*64+d, b*8+oc])
            nc.sync.dma_start(out=out_d, in_=v32[:])
            break

        # ---------- scatter v = t*bc into block-diag vblk (fused) -------
        # vblk[e*64+d, oc, b*16+2oc+e] = t_sb[e*64+d, b*8+oc] * bc[...]
        pitch = vblk[:].ap[0][0]
        for e in range(2):
            dst = bass.AP(tensor=vblk[:].tensor,
                          offset=64 * e * pitch + e,
                          ap=[[pitch, 64], [130, OC], [16, BL]])
            nc.vector.tensor_tensor(
                out=dst,
                in0=t_sb[64 * e:64 * e + 64, :].rearrange(
                    "p (b o) -> p o b", o=OC),
                in1=bc_ps[64 * e:64 * e + 64, 0:64].rearrange(
                    "p (b o) -> p o b", o=OC),
                op=MULT)

        # ---------- P^T = Vblk^T W^T, h-halves pipelined ----------
        pT_ps = ps_big.tile([128, 1024], f32, tag="big", name=f"pT{it}")
        _warm(pT_ps, 3 if it == 0 else 8, big=(it > 0))
        pT_sb = work.tile([128, 1024], f16, tag="pT_sb")
        p_sb = work.tile([128, HC, 128], f16, tag="p_sb")
        for half in range(2):
            hs = slice(512 * half, 512 * half + 512)
            for oc in range(OC):
                nc.tensor.matmul(
                    pT_ps[:, hs],
                    vblk[:, oc, :],
                    wt16[:, oc, hs],
                    start=(oc == 0), stop=(oc == OC - 1),
                    skip_group_check=True)
            _act_copy(nc, pT_sb[:, hs], pT_ps[:, hs])
            _tr(it, p_sb[:, 4 * half:4 * half + 4, :], pT_sb[:, hs],
                f"ptr{it}_{half}")

        # ---------- upd^T = P^T X (col-tiled per batch) ----------
        # 2 groups of 4 batches (4-way col-tiled); each group's softmax
        # tail chain hides under the other group's matmuls / next y-g0.
        u_ps = [ps_u.tile([128, 512], f32, tag="u", name=f"u{it}_{g}")
                for g in range(2)]
        u_sb = work.tile([128, 2, 512], f16, tag="u_sb")
        ut = work.tile([128, 2, SC, 128], f16, tag="ut")
        _warm(u_ps[0], 3 if it == 0 else 6, big=(it > 0))
        for g in range(2):
            nc.tensor.matmul(u_ps[g][:], zeros[:], x16[:, 0, 0, 0:512],
                             start=True, stop=False, skip_group_check=True)

        def _upd_tail(g):
            # transpose (always PE: avoids the ~1.2us DMA completion-sem
            # latency and queue hops), logits add, softmax -> c16
            tp = ps_sm.tile([128, 512], f16, tag="sm", name=f"utp{it}_{g}")
            for k in range(4):
                nc.tensor.matmul(tp[:, 128 * k:128 * k + 128],
                                 u_sb[:, g, 128 * k:128 * k + 128], ident[:],
                                 is_transpose=True, start=(k == 0),
                                 stop=(k == 3), skip_group_check=True)
            _act_copy(nc, ut[:, g],
                      tp[:].rearrange("p (a b) -> p a b", b=128))
            srcu = ut[:, g].rearrange("p sc (b_ jp) -> p sc b_ jp", jp=32)
            nc.vector.tensor_add(
                logits[:, :, 4 * g:4 * g + 4, :],
                logits[:, :, 4 * g:4 * g + 4, :], srcu[:, :, :, 0:16])
            ex = small.tile([128, SC, 4, NCAP], f32, tag="ex")
            nc.scalar.activation(out=ex[:],
                                 in_=logits[:, :, 4 * g:4 * g + 4, :],
                                 func=EXP, scale=1.0, alpha=0.0)
            sm = small.tile([128, SC, 4, 1], f32, tag="sm")
            nc.vector.reduce_sum(sm[:], ex[:], axis=mybir.AxisListType.X)
            rc = small.tile([128, SC, 4, 1], f32, tag="rc")
            nc.vector.reciprocal(rc[:], sm[:])
            nc.vector.tensor_mul(c16[:, :, 4 * g:4 * g + 4, 0:16], ex[:],
                                 rc.broadcast_to([128, SC, 4, NCAP]))

        # hcq-outer so the p_sb-h1 transpose wait hides under g1's
        # hcq0 matmuls.  Evac for g0 is emitted right after its hcq1
        # block (ScalarE runs it under g1's matmuls); the PE-transpose
        # + softmax chains follow both blocks, with the g1 chain
        # hiding under the next iteration's y-g0.
        for hcq in range(2):
            for g in range(2):
                for hc in range(4 * hcq, 4 * hcq + 4):
                    for b_ in range(4):
                        b = 4 * g + b_
                        nc.tensor.matmul(
                            u_ps[g][32 * b_:32 * b_ + 16, :],
                            p_sb[:, hc, 16 * b:16 * b + 16],
                            xt16[:, b, hc, :],
                            start=False, stop=(hc == 7),
                            skip_group_check=True,
                            tile_position=(0, 32 * b_))
                if hcq == 1:
                    _act_copy(nc, u_sb[:, g, :], u_ps[g][:])
        for g in range(2):
            _upd_tail(g)
    ctx.close()


_CACHE = {}


def _host_consts():
    c0 = np.zeros((128, SC, BL, 32), np.float16)
    c0[:, :, :, 0:NCAP] = np.float16(1.0 / NCAP)
    logi = np.zeros((128, SC, BL, NCAP), np.float32)
    vblk0 = np.zeros((128, OC, 128), np.float16)
    ones2 = np.zeros((128, 2), np.float16)
    ones2[0:64, 0] = 1.0
    ones2[64:128, 1] = 1.0
    o2t = np.zeros((2, 128), np.float32)
    o2t[0, 0:64] = -1.0          # sign-fix for the single-Newton rsqrt
    o2t[1, 64:128] = -1.0
    zeros = np.zeros((128, 128), np.float16)
    magic = np.full((2, 1), MAGIC, np.int32)
    ident = np.eye(128, dtype=np.float16)
    return {"c0i": c0, "logi": logi, "vblki": vblk0, "ones2": ones2,
            "o2t": o2t, "zeros": zeros, "magic": magic, "ident": ident}


def _get_nc():
    if "nc" not in _CACHE:
        nc = bacc.Bacc("TRN2", target_bir_lowering=False, debug=False)
        x_d = nc.dram_tensor("x16", [128, BL, SC, 1024], f16,
                             kind="ExternalInput")
        xt_d = nc.dram_tensor("xt16", [128, BL, HC, 512], f16,
                              kind="ExternalInput")
        w_d = nc.dram_tensor("w16", [128, HC, 1024], f16,
                             kind="ExternalInput")
        wt_d = nc.dram_tensor("wt16", [128, OC, 1024], f16,
                              kind="ExternalInput")
        c0_d = nc.dram_tensor("c0i", [128, SC, BL, 32], f16,
                              kind="ExternalInput")
        logits_d = nc.dram_tensor("logi", [128, SC, BL, NCAP], f32,
                                  kind="ExternalInput")
        vblk_d = nc.dram_tensor("vblki", [128, OC, 128], f16,
                                kind="ExternalInput")
        ones2_d = nc.dram_tensor("ones2", [128, 2], f16, kind="ExternalInput")
        o2t_d = nc.dram_tensor("o2t", [2, 128], f32, kind="ExternalInput")
        zeros_d = nc.dram_tensor("zeros", [128, 128], f16,
                                 kind="ExternalInput")
        magic_d = nc.dram_tensor("magic", [2, 1], i32, kind="ExternalInput")
        ident_d = nc.dram_tensor("ident", [128, 128], f16,
                                 kind="ExternalInput")
        out_d = nc.dram_tensor("out", [128, 64], f32, kind="ExternalOutput")
        with tile.TileContext(nc) as tc:
            _build_kernel(tc, out_d.ap(), x_d.ap(), xt_d.ap(), w_d.ap(),
                          wt_d.ap(), c0_d.ap(), logits_d.ap(), vblk_d.ap(),
                          ones2_d.ap(), o2t_d.ap(), zeros_d.ap(),
                          magic_d.ap(), ident_d.ap())
        nc.compile()
        _CACHE["nc"] = nc
    return _CACHE["nc"]


def kernel(inputs: np.ndarray, W: np.ndarray, _trace: bool = False):
    """inputs: (512, 64, 1024) f32; W: (1, 1024, 1024) f32.
    Returns (64, 16, 64) f32."""
    nc = _get_nc()
    consts = _host_consts()
    w0 = W[0].astype(np.float16)
    w16h = np.ascontiguousarray(
        w0.reshape(HC, 128, 1024).transpose(1, 0, 2))
    wt16h = np.ascontiguousarray(
        w0.reshape(1024, OC, 128).transpose(2, 1, 0))
    x16f = inputs.astype(np.float16)              # (S, B, H)
    in_maps = []
    for c in range(N_CORES):
        xs = x16f[:, c * BL:(c + 1) * BL, :]      # (S, BL, H)
        x16h = np.ascontiguousarray(
            xs.reshape(SC, 128, BL, H).transpose(1, 2, 0, 3))
        xt16h = np.ascontiguousarray(
            xs.reshape(S, BL, HC, 128).transpose(3, 1, 2, 0))
        m = {"x16": x16h, "xt16": xt16h, "w16": w16h, "wt16": wt16h}
        m.update(consts)
        in_maps.append(m)
    kw = {}
    if _trace:
        kw = dict(trace=True, trace_cores=[0], stitch_traces=False)
    res = run_bass_kernel_spmd(nc, in_maps, core_ids=list(range(N_CORES)),
                               **kw)
    outs = []
    for c in range(N_CORES):
        v = res.results[c]["out"]          # (128=e*64+d, 64=b*8+oc)
        outs.append(v.reshape(2, 64, BL, 8).transpose(2, 3, 0, 1)
                     .reshape(BL, NCAP, DCAP))
    out = np.concatenate(outs, axis=0)
    if _trace:
        return out.astype(np.float32), res
    return out.astype(np.float32)


# revision 27
# speedup vs baseline: 1.3552x; 1.0037x over previous
"""Trainium2 Bass kernel for capsule dynamic routing (nn_Capsule) — v4.

Reference (per batch item b):
    u = x_b @ W; logits = 0
    for i in 4:
        c = softmax(logits, axis=capsule)
        t_j = sum_s c[s,j] * u[s, j*64:(j+1)*64]; v = squash(t)
        if i < 3: logits[s,j] += u[s, jblk] . v_j

Never materializes u (linearity):
    y_j   = sum_s c[s,j] x_s            y-GEMM   (c stationary, col-tiled)
    t     = W^T y^T                     t-GEMM   (w16 stationary per-slice)
    P^T   = Vblk^T W^T                  P-GEMM   (vblk stationary, block-diag)
    upd^T = P^T X                       upd-GEMM (P slices stationary, col-tiled)

v4 vs v3 (230us): trace showed PE active only 53%, HAM cold 40% of span.
  - queue discipline: ALL bulk input loads ride the scalar HWDGE queue;
    the sync queue carries only tiny consts then mid-iteration
    transposes.  (v3 split loads across both queues, so iteration-0/1
    transposes sat 40+us behind xt16 loads -> PE starved -> HAM cold.)
  - load order w16 -> x16 (per b,half chunks) -> wt16 -> xt16 (per
    b,hcq chunks), matching first-use order; iteration-0 y-GEMM is
    emitted batch-major so it consumes x16 chunks as they land.
  - per-stage emission reordered so every stage-bridging evac/
    transpose/softmax hides under another block's matmuls:
      y: half-outer (iters>=1) with per-(half,g) evac+transpose;
      upd: per-g softmax tail emitted between the two hcq1 g-blocks;
      t: per-oc block-diag extract emitted right after that oc's last
      accumulation matmul.
  - transposes alternate sync/scalar queues once loads are done.
  - warm fillers only where a real >3us PE gap is unavoidable
    (pre-stage), rhs=zeros so they never dep on input DMAs.

HW lessons kept from v3:
  - DVE copy PSUM(f32)->SBUF(f16) kills the device; PSUM->f16 casts go
    through ScalarE activation(Copy).
  - matmul start=True lazily zeroes the whole 2KB PSUM bank: accumulation
    groups must own a (partition-range x bank) region exclusively;
    partition-disjoint groups interleave with skip_group_check=True;
    column-disjoint writes into one bank are fine after the first
    start=True (has_written is per-element).
  - squash rsqrt on DVE (bitcast seed + 2 Newton steps); ScalarE runs
    only Copy+Exp -> exactly ONE ACT table load for the whole kernel.
  - nc.vector.memset on f16 tiles is unreliable: constants come from host.
"""
import numpy as np
from contextlib import ExitStack

import concourse.bass as bass
import concourse.bacc as bacc
import concourse.tile as tile
from concourse import mybir
from concourse.bass_utils import run_bass_kernel_spmd

f16 = mybir.dt.float16
f32 = mybir.dt.float32
i32 = mybir.dt.int32
COPY = mybir.ActivationFunctionType.Copy
EXP = mybir.ActivationFunctionType.Exp
MULT = mybir.AluOpType.mult
SUB = mybir.AluOpType.subtract
ADD = mybir.AluOpType.add
SHR = mybir.AluOpType.logical_shift_right

S, B, H = 512, 64, 1024
NCAP, DCAP = 16, 64
ROUTINGS = 4
N_CORES = 8
BL = B // N_CORES          # 8 batch items per core
SC = S // 128              # 4 s-chunks
HC = H // 128              # 8 h-chunks
OC = H // 128              # 8 o-chunks (o = NCAP*DCAP = 1024)
MAGIC = 0x5EF759DF         # rsqrt seed for h = s/2: 0x5f3759df - (1<<22)


def _act_copy(nc, out, in_):
    nc.scalar.activation(out=out, in_=in_, func=COPY, scale=1.0, alpha=0.0)


def _build_kernel(tc, out_d, x_d, xt_d, w_d, wt_d, c0_d, logits_d, vblk_d,
                  ones2_d, o2t_d, zeros_d, magic_d, ident_d):
    nc = tc.nc
    ctx = ExitStack()
    const = ctx.enter_context(tc.tile_pool(name="const", bufs=1))
    work = ctx.enter_context(tc.tile_pool(name="work", bufs=1))
    small = ctx.enter_context(tc.tile_pool(name="small", bufs=2))
    ps_big = ctx.enter_context(tc.tile_pool(name="ps_big", bufs=2,
                                            space="PSUM"))
    ps_u = ctx.enter_context(tc.tile_pool(name="ps_u", bufs=2, space="PSUM"))
    ps_sm = ctx.enter_context(tc.tile_pool(name="ps_sm", bufs=2,
                                           space="PSUM"))

    # ---------- persistent tensors ----------
    x16 = const.tile([128, BL, SC, 1024], f16)      # (s_loc, b, sc, h)
    xt16 = const.tile([128, BL, HC, 512], f16)      # (h_loc, b, hc, s)
    w16 = const.tile([128, HC, 1024], f16)          # (h_loc, hc, o)
    wt16 = const.tile([128, OC, 1024], f16)         # (o_loc, oc, h)
    c16 = const.tile([128, SC, BL, 32], f16)        # coeffs, cols 16-31 zero
    logits = const.tile([128, SC, BL, NCAP], f32)
    vblk = const.tile([128, OC, 128], f16)          # block-diag v, bj dense
    ones2 = const.tile([128, 2], f16)               # [[1;0],[0;1]] halves
    o2t = const.tile([2, 128], f32)                 # broadcast helper
    zeros = const.tile([128, 128], f16)             # zero-weight psum fill
    magic = const.tile([2, 1], i32)                 # rsqrt seed constant
    ident = const.tile([128, 128], f16)             # PE-transpose identity

    # ---------- loads ----------
    # ALL DMAs (loads + transposes) ride the sync (SP) queue: HWDGE DMA
    # issue blocks the issuing engine for ~0.5-1.5us per call, so the
    # scalar (ACT) engine must never issue DMAs or its evacs stall.
    # Loads are big per-b contiguous chunks in first-use order.
    # zeros+c16 first (warm fillers + y0 weights), then x b0-3 so the
    # iteration-0 y-GEMM starts ASAP; remaining consts ride behind.
    # w16 before x b4-7 (t-GEMM right after y); wt16 before xt16.
    nc.sync.dma_start(out=zeros[:], in_=zeros_d[:])
    nc.sync.dma_start(out=c16[:], in_=c0_d[:])
    for b in range(4):
        nc.sync.dma_start(out=x16[:, b], in_=x_d[:, b])
    nc.sync.dma_start(out=ident[:], in_=ident_d[:])
    nc.sync.dma_start(out=ones2[:], in_=ones2_d[:])
    nc.sync.dma_start(out=o2t[:], in_=o2t_d[:])
    nc.sync.dma_start(out=magic[:], in_=magic_d[:])
    nc.sync.dma_start(out=logits[:], in_=logits_d[:])
    nc.sync.dma_start(out=vblk[:], in_=vblk_d[:])
    nc.sync.dma_start(out=w16[:], in_=w_d[:])
    for b in range(4, BL):
        nc.sync.dma_start(out=x16[:, b], in_=x_d[:, b])
    nc.sync.dma_start(out=wt16[:], in_=wt_d[:])
    for b in range(BL):
        nc.sync.dma_start(out=xt16[:, b], in_=xt_d[:, b])

    def _warm(ps_tile, n, big=False):
        # zero-weight matmuls into a psum tile whose next real producer
        # begins with start=True (which wipes the bank): pure HAM fuel
        # that runs during the dependency-wait gap before the stage.
        # big=True streams x16 (N=512, 215ns each) for post-load fillers;
        # otherwise rhs=zeros so fillers never wait on input DMAs.
        for k in range(n):
            if big:
                nc.tensor.matmul(ps_tile[:, 0:512], zeros[:],
                                 x16[:, 0, 0, 0:512],
                                 start=(k == 0), stop=False,
                                 skip_group_check=True)
            else:
                nc.tensor.matmul(ps_tile[:, 0:128], zeros[:], zeros[:],
                                 start=(k == 0), stop=False,
                                 skip_group_check=True)

    def _tr(it, out3d, in2d, name):
        # SBUF transpose.  Iteration 0 uses PE transpose-mode (the HWDGE
        # transpose barrier + shared sem pool would serialize DMA
        # transposes against the in-flight input loads, stalling both);
        # PE is idle during the load window anyway and this keeps HAM
        # warm.  Iterations >=1 use xbar DMA transposes on sync.
        if it == 0:
            n = in2d.free_size() // 128
            tp = ps_sm.tile([128, 512], f16, tag="sm", name=name)
            for k in range(n):
                nc.tensor.matmul(tp[:, 128 * k:128 * k + 128],
                                 in2d[:, 128 * k:128 * k + 128], ident[:],
                                 is_transpose=True, start=(k == 0),
                                 stop=(k == n - 1), skip_group_check=True)
            _act_copy(nc, out3d,
                      tp[:, 0:128 * n].rearrange("p (a b) -> p a b", b=128))
        else:
            nc.sync.dma_start_transpose(out3d, in2d)

    for it in range(ROUTINGS):
        last = it == ROUTINGS - 1

        # ---------- y = C^T X ----------
        y_ps = [ps_big.tile([128, 1024], f32, tag="big", name=f"y{it}_{g}")
                for g in range(2)]
        if it == 0:
            _warm(y_ps[0], 16)
            import os
            if os.environ.get("DIAG_FILL"):
                _warm(y_ps[0], 300, big=True)
        else:
            # cover the tail-g1 softmax chain + keep HAM warm
            _warm(y_ps[0], 8, big=True)
        y_sb = work.tile([128, 2, 1024], f16, tag="y_sb")
        yt = work.tile([128, HC, 256], f16, tag="yt")

        def _y_mm(g, b_, half, sc):
            b = 4 * g + b_
            hs = slice(512 * half, 512 * half + 512)
            nc.tensor.matmul(
                y_ps[g][32 * b_:32 * b_ + 32, hs],
                c16[:, sc, b, :],
                x16[:, b, sc, hs],
                start=(sc == 0), stop=(sc == SC - 1),
                skip_group_check=True,
                tile_position=(0, 32 * b_))

        def _y_evac(g, half):
            hs = slice(512 * half, 512 * half + 512)
            _act_copy(nc, y_sb[:, g, hs], y_ps[g][:, hs])
            _tr(it, yt[:, 4 * half:4 * half + 4, 128 * g:128 * g + 128],
                y_sb[:, g, hs], f"ytr{it}_{half}_{g}")

        if it == 0:
            # batch-major: track x16 arrival order b0..b7
            for g in range(2):
                for b_ in range(4):
                    for half in range(2):
                        for sc in range(SC):
                            _y_mm(g, b_, half, sc)
                for half in range(2):
                    _y_evac(g, half)
        else:
            # g-outer: y-g0 runs while upd tails s2/s3 finish; y-g1's
            # c16 is ready by the time y-g0's matmuls complete
            for g in range(2):
                for half in range(2):
                    for b_ in range(4):
                        for sc in range(SC):
                            _y_mm(g, b_, half, sc)
                    _y_evac(g, half)

        # ---------- t = W^T y^T, two hc passes ----------
        yt_dense = yt.rearrange("p hc (g b_ jp) -> p hc g b_ jp", g=2, jp=32)
        t_ps = ps_big.tile([128, 1024], f32, tag="big", name=f"t{it}")
        _warm(t_ps, 2 if it == 0 else 4, big=(it > 0))
        for hcq in range(2):
            for oc in range(OC):
                for hc in range(4 * hcq, 4 * hcq + 4):
                    nc.tensor.matmul(
                        t_ps[:, oc * 128:oc * 128 + 128],
                        w16[:, hc, oc * 128:oc * 128 + 128],
                        yt_dense[:, hc, :, :, 0:16],
                        start=(hcq == 0 and hc == 0 and oc % 4 == 0),
                        stop=(hcq == 1 and hc == 7),
                        skip_group_check=True)
            if hcq == 0 and it > 0:
                # bridge the yt-h1 transpose wait before the hcq1 pass
                fb = ps_sm.tile([128, 512], f32, tag="sm", name=f"tf{it}")
                _warm(fb, 5, big=True)

        # ---------- extract block-diag: t_sb (e*64+d, b*8+oc) ----------
        # t_ps col for (oc, b, j=2oc+e) = oc*130 + b*16 + e: linear in
        # (oc, b), so one strided copy per e-half (DVE + ScalarE in
        # parallel) replaces 16 tiny per-oc copies.
        t_sb = small.tile([128, 64], f32, tag="t_sb")
        pps = t_ps[:].ap[0][0]
        psb = t_sb[:].ap[0][0]
        for e in range(2):
            srcd = bass.AP(tensor=t_ps[:].tensor, offset=64 * e * pps + e,
                           ap=[[pps, 64], [130, OC], [16, BL]])
            dstd = bass.AP(tensor=t_sb[:].tensor, offset=64 * e * psb,
                           ap=[[psb, 64], [1, OC], [8, BL]])
            if e == 0:
                nc.vector.tensor_copy(dstd, srcd)
            else:
                _act_copy(nc, dstd, srcd)

        # ---------- squash: rs = rsqrt(sum_d t^2 + eps) on DVE ----------
        t2 = small.tile([128, 64], f16, tag="t2")
        nc.vector.tensor_mul(t2[:], t_sb[:], t_sb[:])
        sq_ps = ps_sm.tile([2, 512], f32, tag="sm", name=f"sq{it}")
        nc.tensor.matmul(sq_ps[:, 0:64], ones2[:], t2[:])
        h_sb = small.tile([2, 64], f32, tag="h_sb")
        nc.vector.tensor_scalar(out=h_sb[:], in0=sq_ps[:, 0:64],
                                scalar1=0.5, scalar2=5e-8, op0=MULT, op1=ADD)
        ri = small.tile([2, 64], i32, tag="ri")
        nc.vector.tensor_scalar(out=ri[:], in0=h_sb.bitcast(i32),
                                scalar1=1, scalar2=None, op0=SHR)
        r0 = small.tile([2, 64], f32, tag="r0")
        nc.vector.tensor_tensor(out=r0.bitcast(i32),
                                in0=magic.broadcast_to([2, 64]),
                                in1=ri[:], op=SUB)
        # Newton x1 with sign fold: rs = (h*r0*r0 - 1.5) * r0 = -rsqrt
        # approx (0.2% err, fine at 2e-2 tol); o2t carries -1 so the
        # broadcast flips the sign back.
        rr = small.tile([2, 64], f32, tag="rr")
        rs = small.tile([2, 64], f32, tag="rs")
        nc.vector.tensor_mul(rr[:], r0[:], r0[:])
        nc.vector.tensor_mul(rr[:], rr[:], h_sb[:])
        nc.vector.scalar_tensor_tensor(out=rs[:], in0=rr[:], scalar=1.5,
                                       in1=r0[:], op0=SUB, op1=MULT)
        # broadcast -rs (2,64) -> (128,64): bc[p,n] = -rs[p//64, n]
        bc_ps = ps_sm.tile([128, 512], f32, tag="sm", name=f"bc{it}")
        nc.tensor.matmul(bc_ps[:, 0:64], o2t[:], rs[:])

        if last:
            v32 = small.tile([128, 64], f32, tag="v32")
            nc.vector.tensor_mul(v32[:], t_sb[:], bc_ps[:, 0:64])
            # raw dump; host reorders (out[b,2oc+e,d] = v32[e*64+d, b*8+oc])
            nc.sync.dma_start(out=out_d, in_=v32[:])
            break

        # ---------- scatter v = t*bc into block-diag vblk (fused) -------
        # vblk[e*64+d, oc, b*16+2oc+e] = t_sb[e*64+d, b*8+oc] * bc[...]
        pitch = vblk[:].ap[0][0]
        for e in range(2):
            dst = bass.AP(tensor=vblk[:].tensor,
                          offset=64 * e * pitch + e,
                          ap=[[pitch, 64], [130, OC], [16, BL]])
            nc.vector.tensor_tensor(
                out=dst,
                in0=t_sb[64 * e:64 * e + 64, :].rearrange(
                    "p (b o) -> p o b", o=OC),
                in1=bc_ps[64 * e:64 * e + 64, 0:64].rearrange(
                    "p (b o) -> p o b", o=OC),
                op=MULT)

        # ---------- P^T = Vblk^T W^T, h-halves pipelined ----------
        pT_ps = ps_big.tile([128, 1024], f32, tag="big", name=f"pT{it}")
        _warm(pT_ps, 3 if it == 0 else 8, big=(it > 0))
        pT_sb = work.tile([128, 1024], f16, tag="pT_sb")
        p_sb = work.tile([128, HC, 128], f16, tag="p_sb")
        for half in range(2):
            hs = slice(512 * half, 512 * half + 512)
            for oc in range(OC):
                nc.tensor.matmul(
                    pT_ps[:, hs],
                    vblk[:, oc, :],
                    wt16[:, oc, hs],
                    start=(oc == 0), stop=(oc == OC - 1),
                    skip_group_check=True)
            _act_copy(nc, pT_sb[:, hs], pT_ps[:, hs])
            _tr(it, p_sb[:, 4 * half:4 * half + 4, :], pT_sb[:, hs],
                f"ptr{it}_{half}")

        # ---------- upd^T = P^T X (col-tiled per batch) ----------
        # 2 groups of 4 batches (4-way col-tiled); each group's softmax
        # tail chain hides under the other group's matmuls / next y-g0.
        u_ps = [ps_u.tile([128, 512], f32, tag="u", name=f"u{it}_{g}")
                for g in range(2)]
        u_sb = work.tile([128, 2, 512], f16, tag="u_sb")
        ut = work.tile([128, 2, SC, 128], f16, tag="ut")
        _warm(u_ps[0], 3 if it == 0 else 6, big=(it > 0))
        for g in range(2):
            nc.tensor.matmul(u_ps[g][:], zeros[:], x16[:, 0, 0, 0:512],
                             start=True, stop=False, skip_group_check=True)

        def _upd_tail(g):
            # transpose (always PE: avoids the ~1.2us DMA completion-sem
            # latency and queue hops), logits add, softmax -> c16
            tp = ps_sm.tile([128, 512], f16, tag="sm", name=f"utp{it}_{g}")
            for k in range(4):
                nc.tensor.matmul(tp[:, 128 * k:128 * k + 128],
                                 u_sb[:, g, 128 * k:128 * k + 128], ident[:],
                                 is_transpose=True, start=(k == 0),
                                 stop=(k == 3), skip_group_check=True)
            _act_copy(nc, ut[:, g],
                      tp[:].rearrange("p (a b) -> p a b", b=128))
            srcu = ut[:, g].rearrange("p sc (b_ jp) -> p sc b_ jp", jp=32)
            nc.vector.tensor_add(
                logits[:, :, 4 * g:4 * g + 4, :],
                logits[:, :, 4 * g:4 * g + 4, :], srcu[:, :, :, 0:16])
            ex = small.tile([128, SC, 4, NCAP], f32, tag="ex")
            nc.scalar.activation(out=ex[:],
                                 in_=logits[:, :, 4 * g:4 * g + 4, :],
                                 func=EXP, scale=1.0, alpha=0.0)
            sm = small.tile([128, SC, 4, 1], f32, tag="sm")
            nc.vector.reduce_sum(sm[:], ex[:], axis=mybir.AxisListType.X)
            rc = small.tile([128, SC, 4, 1], f32, tag="rc")
            nc.vector.reciprocal(rc[:], sm[:])
            nc.vector.tensor_mul(c16[:, :, 4 * g:4 * g + 4, 0:16], ex[:],
                                 rc.broadcast_to([128, SC, 4, NCAP]))

        # hcq-outer so the p_sb-h1 transpose wait hides under g1's
        # hcq0 matmuls.  Evac for g0 is emitted right after its hcq1
        # block (ScalarE runs it under g1's matmuls); the PE-transpose
        # + softmax chains follow both blocks, with the g1 chain
        # hiding under the next iteration's y-g0.
        for hcq in range(2):
            for g in range(2):
                for hc in range(4 * hcq, 4 * hcq + 4):
                    for b_ in range(4):
                        b = 4 * g + b_
                        nc.tensor.matmul(
                            u_ps[g][32 * b_:32 * b_ + 16, :],
                            p_sb[:, hc, 16 * b:16 * b + 16],
                            xt16[:, b, hc, :],
                            start=False, stop=(hc == 7),
                            skip_group_check=True,
                            tile_position=(0, 32 * b_))
                if hcq == 1:
                    _act_copy(nc, u_sb[:, g, :], u_ps[g][:])
        for g in range(2):
            _upd_tail(g)
    ctx.close()


_CACHE = {}


def _host_consts():
    c0 = np.zeros((128, SC, BL, 32), np.float16)
    c0[:, :, :, 0:NCAP] = np.float16(1.0 / NCAP)
    logi = np.zeros((128, SC, BL, NCAP), np.float32)
    vblk0 = np.zeros((128, OC, 128), np.float16)
    ones2 = np.zeros((128, 2), np.float16)
    ones2[0:64, 0] = 1.0
    ones2[64:128, 1] = 1.0
    o2t = np.zeros((2, 128), np.float32)
    o2t[0, 0:64] = -1.0          # sign-fix for the single-Newton rsqrt
    o2t[1, 64:128] = -1.0
    zeros = np.zeros((128, 128), np.float16)
    magic = np.full((2, 1), MAGIC, np.int32)
    ident = np.eye(128, dtype=np.float16)
    return {"c0i": c0, "logi": logi, "vblki": vblk0, "ones2": ones2,
            "o2t": o2t, "zeros": zeros, "magic": magic, "ident": ident}


def _get_nc():
    if "nc" not in _CACHE:
        nc = bacc.Bacc("TRN2", target_bir_lowering=False, debug=False)
        x_d = nc.dram_tensor("x16", [128, BL, SC, 1024], f16,
                             kind="ExternalInput")
        xt_d = nc.dram_tensor("xt16", [128, BL, HC, 512], f16,
                              kind="ExternalInput")
        w_d = nc.dram_tensor("w16", [128, HC, 1024], f16,
                             kind="ExternalInput")
        wt_d = nc.dram_tensor("wt16", [128, OC, 1024], f16,
                              kind="ExternalInput")
        c0_d = nc.dram_tensor("c0i", [128, SC, BL, 32], f16,
                              kind="ExternalInput")
        logits_d = nc.dram_tensor("logi", [128, SC, BL, NCAP], f32,
                                  kind="ExternalInput")
        vblk_d = nc.dram_tensor("vblki", [128, OC, 128], f16,
                                kind="ExternalInput")
        ones2_d = nc.dram_tensor("ones2", [128, 2], f16, kind="ExternalInput")
        o2t_d = nc.dram_tensor("o2t", [2, 128], f32, kind="ExternalInput")
        zeros_d = nc.dram_tensor("zeros", [128, 128], f16,
                                 kind="ExternalInput")
        magic_d = nc.dram_tensor("magic", [2, 1], i32, kind="ExternalInput")
        ident_d = nc.dram_tensor("ident", [128, 128], f16,
                                 kind="ExternalInput")
        out_d = nc.dram_tensor("out", [128, 64], f32, kind="ExternalOutput")
        with tile.TileContext(nc) as tc:
            _build_kernel(tc, out_d.ap(), x_d.ap(), xt_d.ap(), w_d.ap(),
                          wt_d.ap(), c0_d.ap(), logits_d.ap(), vblk_d.ap(),
                          ones2_d.ap(), o2t_d.ap(), zeros_d.ap(),
                          magic_d.ap(), ident_d.ap())
        nc.compile()
        _CACHE["nc"] = nc
    return _CACHE["nc"]


def kernel(inputs: np.ndarray, W: np.ndarray, _trace: bool = False):
    """inputs: (512, 64, 1024) f32; W: (1, 1024, 1024) f32.
    Returns (64, 16, 64) f32."""
    nc = _get_nc()
    consts = _host_consts()
    w0 = W[0].astype(np.float16)
    w16h = np.ascontiguousarray(
        w0.reshape(HC, 128, 1024).transpose(1, 0, 2))
    wt16h = np.ascontiguousarray(
        w0.reshape(1024, OC, 128).transpose(2, 1, 0))
    x16f = inputs.astype(np.float16)              # (S, B, H)
    in_maps = []
    for c in range(N_CORES):
        xs = x16f[:, c * BL:(c + 1) * BL, :]      # (S, BL, H)
        x16h = np.ascontiguousarray(
            xs.reshape(SC, 128, BL, H).transpose(1, 2, 0, 3))
        xt16h = np.ascontiguousarray(
            xs.reshape(S, BL, HC, 128).transpose(3, 1, 2, 0))
        m = {"x16": x16h, "xt16": xt16h, "w16": w16h, "wt16": wt16h}
        m.update(consts)
        in_maps.append(m)
    kw = {}
    if _trace:
        kw = dict(trace=True, trace_cores=[0], stitch_traces=False)
    res = run_bass_kernel_spmd(nc, in_maps, core_ids=list(range(N_CORES)),
                               **kw)
    outs = []
    for c in range(N_CORES):
        v = res.results[c]["out"]          # (128=e*64+d, 64=b*8+oc)
        outs.append(v.reshape(2, 64, BL, 8).transpose(2, 3, 0, 1)
                     .reshape(BL, NCAP, DCAP))
    out = np.concatenate(outs, axis=0)
    if _trace:
        return out.astype(np.float32), res
    return out.astype(np.float32)


# revision 31
# speedup vs baseline: 1.4730x; 1.0869x over previous
"""Trainium2 Bass kernel for capsule dynamic routing (nn_Capsule) — v4.

Reference (per batch item b):
    u = x_b @ W; logits = 0
    for i in 4:
        c = softmax(logits, axis=capsule)
        t_j = sum_s c[s,j] * u[s, j*64:(j+1)*64]; v = squash(t)
        if i < 3: logits[s,j] += u[s, jblk] . v_j

Never materializes u (linearity):
    y_j   = sum_s c[s,j] x_s            y-GEMM   (c stationary, col-tiled)
    t     = W^T y^T                     t-GEMM   (w16 stationary per-slice)
    P^T   = Vblk^T W^T                  P-GEMM   (vblk stationary, block-diag)
    upd^T = P^T X                       upd-GEMM (P slices stationary, col-tiled)

v4 vs v3 (230us): trace showed PE active only 53%, HAM cold 40% of span.
  - queue discipline: ALL bulk input loads ride the scalar HWDGE queue;
    the sync queue carries only tiny consts then mid-iteration
    transposes.  (v3 split loads across both queues, so iteration-0/1
    transposes sat 40+us behind xt16 loads -> PE starved -> HAM cold.)
  - load order w16 -> x16 (per b,half chunks) -> wt16 -> xt16 (per
    b,hcq chunks), matching first-use order; iteration-0 y-GEMM is
    emitted batch-major so it consumes x16 chunks as they land.
  - per-stage emission reordered so every stage-bridging evac/
    transpose/softmax hides under another block's matmuls:
      y: half-outer (iters>=1) with per-(half,g) evac+transpose;
      upd: per-g softmax tail emitted between the two hcq1 g-blocks;
      t: per-oc block-diag extract emitted right after that oc's last
      accumulation matmul.
  - transposes alternate sync/scalar queues once loads are done.
  - warm fillers only where a real >3us PE gap is unavoidable
    (pre-stage), rhs=zeros so they never dep on input DMAs.

HW lessons kept from v3:
  - DVE copy PSUM(f32)->SBUF(f16) kills the device; PSUM->f16 casts go
    through ScalarE activation(Copy).
  - matmul start=True lazily zeroes the whole 2KB PSUM bank: accumulation
    groups must own a (partition-range x bank) region exclusively;
    partition-disjoint groups interleave with skip_group_check=True;
    column-disjoint writes into one bank are fine after the first
    start=True (has_written is per-element).
  - squash rsqrt on DVE (bitcast seed + 2 Newton steps); ScalarE runs
    only Copy+Exp -> exactly ONE ACT table load for the whole kernel.
  - nc.vector.memset on f16 tiles is unreliable: constants come from host.
"""
import numpy as np
from contextlib import ExitStack

import concourse.bass as bass
import concourse.bacc as bacc
import concourse.tile as tile
from concourse import mybir
from concourse.bass_utils import run_bass_kernel_spmd

f16 = mybir.dt.float16
f32 = mybir.dt.float32
i32 = mybir.dt.int32
COPY = mybir.ActivationFunctionType.Copy
EXP = mybir.ActivationFunctionType.Exp
MULT = mybir.AluOpType.mult
SUB = mybir.AluOpType.subtract
ADD = mybir.AluOpType.add
SHR = mybir.AluOpType.logical_shift_right

S, B, H = 512, 64, 1024
NCAP, DCAP = 16, 64
ROUTINGS = 4
N_CORES = 8
BL = B // N_CORES          # 8 batch items per core
SC = S // 128              # 4 s-chunks
HC = H // 128              # 8 h-chunks
OC = H // 128              # 8 o-chunks (o = NCAP*DCAP = 1024)
MAGIC = 0x5EF759DF         # rsqrt seed for h = s/2: 0x5f3759df - (1<<22)


def _act_copy(nc, out, in_):
    nc.scalar.activation(out=out, in_=in_, func=COPY, scale=1.0, alpha=0.0)


def _build_kernel(tc, out_d, x_d, xt_d, w_d, wt_d, c0_d, logits_d, vblk_d,
                  ones2_d, o2t_d, zeros_d, magic_d, ident_d):
    nc = tc.nc
    ctx = ExitStack()
    const = ctx.enter_context(tc.tile_pool(name="const", bufs=1))
    work = ctx.enter_context(tc.tile_pool(name="work", bufs=1))
    small = ctx.enter_context(tc.tile_pool(name="small", bufs=2))
    ps_big = ctx.enter_context(tc.tile_pool(name="ps_big", bufs=2,
                                            space="PSUM"))
    ps_u = ctx.enter_context(tc.tile_pool(name="ps_u", bufs=2, space="PSUM"))
    ps_sm = ctx.enter_context(tc.tile_pool(name="ps_sm", bufs=2,
                                           space="PSUM"))

    # ---------- persistent tensors ----------
    x16 = const.tile([128, BL, SC, 1024], f16)      # (s_loc, b, sc, h)
    xt16 = const.tile([128, BL, HC, 512], f16)      # (h_loc, b, hc, s)
    w16 = const.tile([128, HC, 1024], f16)          # (h_loc, hc, o)
    wt16 = const.tile([128, OC, 1024], f16)         # (o_loc, oc, h)
    c16 = const.tile([128, SC, BL, 32], f16)        # coeffs, cols 16-31 zero
    logits = const.tile([128, SC, BL, NCAP], f32)
    vblk = const.tile([128, OC, 128], f16)          # block-diag v, bj dense
    ones2 = const.tile([128, 2], f16)               # [[1;0],[0;1]] halves
    o2t = const.tile([2, 128], f32)                 # broadcast helper
    zeros = const.tile([128, 128], f16)             # zero-weight psum fill
    magic = const.tile([2, 1], i32)                 # rsqrt seed constant
    ident = const.tile([128, 128], f16)             # PE-transpose identity

    # ---------- loads ----------
    # ALL DMAs (loads + transposes) ride the sync (SP) queue: HWDGE DMA
    # issue blocks the issuing engine for ~0.5-1.5us per call, so the
    # scalar (ACT) engine must never issue DMAs or its evacs stall.
    # Loads are big per-b contiguous chunks in first-use order.
    # zeros+c16 first (warm fillers + y0 weights), then x b0-3 so the
    # iteration-0 y-GEMM starts ASAP; remaining consts ride behind.
    # w16 before x b4-7 (t-GEMM right after y); wt16 before xt16.
    nc.sync.dma_start(out=zeros[:], in_=zeros_d[:])
    nc.sync.dma_start(out=c16[:], in_=c0_d[:])
    for b in range(4):
        nc.sync.dma_start(out=x16[:, b], in_=x_d[:, b])
    nc.sync.dma_start(out=ident[:], in_=ident_d[:])
    nc.sync.dma_start(out=ones2[:], in_=ones2_d[:])
    nc.sync.dma_start(out=o2t[:], in_=o2t_d[:])
    nc.sync.dma_start(out=magic[:], in_=magic_d[:])
    nc.sync.dma_start(out=logits[:], in_=logits_d[:])
    nc.sync.dma_start(out=vblk[:], in_=vblk_d[:])
    nc.sync.dma_start(out=w16[:], in_=w_d[:])
    for b in range(4, BL):
        nc.sync.dma_start(out=x16[:, b], in_=x_d[:, b])
    nc.sync.dma_start(out=wt16[:], in_=wt_d[:])
    for b in range(BL):
        nc.sync.dma_start(out=xt16[:, b], in_=xt_d[:, b])

    def _warm(ps_tile, n, big=False):
        # zero-weight matmuls into a psum tile whose next real producer
        # begins with start=True (which wipes the bank): pure HAM fuel
        # that runs during the dependency-wait gap before the stage.
        # big=True streams x16 (N=512, 215ns each) for post-load fillers;
        # otherwise rhs=zeros so fillers never wait on input DMAs.
        for k in range(n):
            if big:
                nc.tensor.matmul(ps_tile[:, 0:512], zeros[:],
                                 x16[:, 0, 0, 0:512],
                                 start=(k == 0), stop=False,
                                 skip_group_check=True)
            else:
                nc.tensor.matmul(ps_tile[:, 0:128], zeros[:], zeros[:],
                                 start=(k == 0), stop=False,
                                 skip_group_check=True)

    def _tr(it, out3d, in2d, name):
        # SBUF transpose.  Iteration 0 uses PE transpose-mode (the HWDGE
        # transpose barrier + shared sem pool would serialize DMA
        # transposes against the in-flight input loads, stalling both);
        # PE is idle during the load window anyway and this keeps HAM
        # warm.  Iterations >=1 use xbar DMA transposes on sync.
        if it == 0:
            n = in2d.free_size() // 128
            tp = ps_sm.tile([128, 512], f16, tag="sm", name=name)
            for k in range(n):
                nc.tensor.matmul(tp[:, 128 * k:128 * k + 128],
                                 in2d[:, 128 * k:128 * k + 128], ident[:],
                                 is_transpose=True, start=(k == 0),
                                 stop=(k == n - 1), skip_group_check=True)
            _act_copy(nc, out3d,
                      tp[:, 0:128 * n].rearrange("p (a b) -> p a b", b=128))
        else:
            nc.sync.dma_start_transpose(out3d, in2d)

    for it in range(ROUTINGS):
        last = it == ROUTINGS - 1

        # ---------- y = C^T X ----------
        y_ps = [ps_big.tile([128, 1024], f32, tag="big", name=f"y{it}_{g}")
                for g in range(2)]
        if it == 0:
            _warm(y_ps[0], 16)
            import os
            if os.environ.get("DIAG_FILL"):
                _warm(y_ps[0], 300, big=True)
        else:
            # cover the tail-g1 softmax chain + keep HAM warm
            _warm(y_ps[0], 8, big=True)
        y_sb = work.tile([128, 2, 1024], f16, tag="y_sb")
        yt = work.tile([128, HC, 256], f16, tag="yt")

        def _y_mm(g, b_, half, sc):
            b = 4 * g + b_
            hs = slice(512 * half, 512 * half + 512)
            nc.tensor.matmul(
                y_ps[g][32 * b_:32 * b_ + 32, hs],
                c16[:, sc, b, :],
                x16[:, b, sc, hs],
                start=(sc == 0), stop=(sc == SC - 1),
                skip_group_check=True,
                tile_position=(0, 32 * b_))

        def _y_evac(g, half):
            hs = slice(512 * half, 512 * half + 512)
            _act_copy(nc, y_sb[:, g, hs], y_ps[g][:, hs])
            _tr(it, yt[:, 4 * half:4 * half + 4, 128 * g:128 * g + 128],
                y_sb[:, g, hs], f"ytr{it}_{half}_{g}")

        if it == 0:
            # batch-major: track x16 arrival order b0..b7
            for g in range(2):
                for b_ in range(4):
                    for half in range(2):
                        for sc in range(SC):
                            _y_mm(g, b_, half, sc)
                for half in range(2):
                    _y_evac(g, half)
        else:
            # half-outer: both h0 transposes complete during the h1
            # matmuls, so t-hcq0 starts right after y.  (Both groups'
            # softmax tails run in parallel across engines, so c16-g1
            # is ready in time.)
            for half in range(2):
                for g in range(2):
                    for b_ in range(4):
                        for sc in range(SC):
                            _y_mm(g, b_, half, sc)
                    _y_evac(g, half)

        # ---------- t = W^T y^T, two hc passes ----------
        yt_dense = yt.rearrange("p hc (g b_ jp) -> p hc g b_ jp", g=2, jp=32)
        t_ps = ps_big.tile([128, 1024], f32, tag="big", name=f"t{it}")
        _warm(t_ps, 2 if it == 0 else 4, big=(it > 0))
        for hcq in range(2):
            for oc in range(OC):
                for hc in range(4 * hcq, 4 * hcq + 4):
                    nc.tensor.matmul(
                        t_ps[:, oc * 128:oc * 128 + 128],
                        w16[:, hc, oc * 128:oc * 128 + 128],
                        yt_dense[:, hc, :, :, 0:16],
                        start=(hcq == 0 and hc == 0 and oc % 4 == 0),
                        stop=(hcq == 1 and hc == 7),
                        skip_group_check=True)


        # ---------- extract block-diag: t_sb (e*64+d, b*8+oc) ----------
        # t_ps col for (oc, b, j=2oc+e) = oc*130 + b*16 + e: linear in
        # (oc, b), so one strided copy per e-half (DVE + ScalarE in
        # parallel) replaces 16 tiny per-oc copies.
        t_sb = small.tile([128, 64], f32, tag="t_sb")
        pps = t_ps[:].ap[0][0]
        psb = t_sb[:].ap[0][0]
        for e in range(2):
            srcd = bass.AP(tensor=t_ps[:].tensor, offset=64 * e * pps + e,
                           ap=[[pps, 64], [130, OC], [16, BL]])
            dstd = bass.AP(tensor=t_sb[:].tensor, offset=64 * e * psb,
                           ap=[[psb, 64], [1, OC], [8, BL]])
            if e == 0:
                nc.vector.tensor_copy(dstd, srcd)
            else:
                _act_copy(nc, dstd, srcd)

        # ---------- squash: rs = rsqrt(sum_d t^2 + eps) on DVE ----------
        t2 = small.tile([128, 64], f16, tag="t2")
        nc.vector.tensor_mul(t2[:], t_sb[:], t_sb[:])
        sq_ps = ps_sm.tile([2, 512], f32, tag="sm", name=f"sq{it}")
        nc.tensor.matmul(sq_ps[:, 0:64], ones2[:], t2[:])
        h_sb = small.tile([2, 64], f32, tag="h_sb")
        nc.vector.tensor_scalar(out=h_sb[:], in0=sq_ps[:, 0:64],
                                scalar1=0.5, scalar2=5e-8, op0=MULT, op1=ADD)
        ri = small.tile([2, 64], i32, tag="ri")
        nc.vector.tensor_scalar(out=ri[:], in0=h_sb.bitcast(i32),
                                scalar1=1, scalar2=None, op0=SHR)
        r0 = small.tile([2, 64], f32, tag="r0")
        nc.vector.tensor_tensor(out=r0.bitcast(i32),
                                in0=magic.broadcast_to([2, 64]),
                                in1=ri[:], op=SUB)
        # Newton x1 with sign fold: rs = (h*r0*r0 - 1.5) * r0 = -rsqrt
        # approx (0.2% err, fine at 2e-2 tol); o2t carries -1 so the
        # broadcast flips the sign back.
        rr = small.tile([2, 64], f32, tag="rr")
        rs = small.tile([2, 64], f32, tag="rs")
        nc.vector.tensor_mul(rr[:], r0[:], r0[:])
        nc.vector.tensor_mul(rr[:], rr[:], h_sb[:])
        nc.vector.scalar_tensor_tensor(out=rs[:], in0=rr[:], scalar=1.5,
                                       in1=r0[:], op0=SUB, op1=MULT)
        # broadcast -rs (2,64) -> (128,64): bc[p,n] = -rs[p//64, n]
        bc_ps = ps_sm.tile([128, 512], f32, tag="sm", name=f"bc{it}")
        nc.tensor.matmul(bc_ps[:, 0:64], o2t[:], rs[:])

        if last:
            v32 = small.tile([128, 64], f32, tag="v32")
            nc.vector.tensor_mul(v32[:], t_sb[:], bc_ps[:, 0:64])
            # raw dump; host reorders (out[b,2oc+e,d] = v32[e*64+d, b*8+oc])
            nc.sync.dma_start(out=out_d, in_=v32[:])
            break

        # ---------- scatter v = t*bc into block-diag vblk (fused) -------
        # vblk[e*64+d, oc, b*16+2oc+e] = t_sb[e*64+d, b*8+oc] * bc[...]
        pitch = vblk[:].ap[0][0]
        for e in range(2):
            dst = bass.AP(tensor=vblk[:].tensor,
                          offset=64 * e * pitch + e,
                          ap=[[pitch, 64], [130, OC], [16, BL]])
            nc.vector.tensor_tensor(
                out=dst,
                in0=t_sb[64 * e:64 * e + 64, :].rearrange(
                    "p (b o) -> p o b", o=OC),
                in1=bc_ps[64 * e:64 * e + 64, 0:64].rearrange(
                    "p (b o) -> p o b", o=OC),
                op=MULT)

        # ---------- P^T = Vblk^T W^T, h-halves pipelined ----------
        pT_ps = ps_big.tile([128, 1024], f32, tag="big", name=f"pT{it}")
        _warm(pT_ps, 3 if it == 0 else 8, big=(it > 0))
        pT_sb = work.tile([128, 1024], f16, tag="pT_sb")
        p_sb = work.tile([128, HC, 128], f16, tag="p_sb")
        for half in range(2):
            hs = slice(512 * half, 512 * half + 512)
            for oc in range(OC):
                nc.tensor.matmul(
                    pT_ps[:, hs],
                    vblk[:, oc, :],
                    wt16[:, oc, hs],
                    start=(oc == 0), stop=(oc == OC - 1),
                    skip_group_check=True)
            _act_copy(nc, pT_sb[:, hs], pT_ps[:, hs])
            if half == 0:
                # h0 rides the xbar DMA (overlaps the h1 matmuls); h1
                # goes PE-transpose, emitted between the upd hcq blocks
                _tr(it, p_sb[:, 0:4, :], pT_sb[:, hs], f"ptr{it}_0")

        # ---------- upd^T = P^T X (col-tiled per batch) ----------
        # 2 groups of 4 batches (4-way col-tiled); each group's softmax
        # tail chain hides under the other group's matmuls / next y-g0.
        u_ps = [ps_u.tile([128, 512], f32, tag="u", name=f"u{it}_{g}")
                for g in range(2)]
        u_sb = work.tile([128, 2, 512], f16, tag="u_sb")
        ut = work.tile([128, 2, SC, 128], f16, tag="ut")
        _warm(u_ps[0], 3 if it == 0 else 6, big=(it > 0))
        for g in range(2):
            nc.tensor.matmul(u_ps[g][:], zeros[:], x16[:, 0, 0, 0:512],
                             start=True, stop=False, skip_group_check=True)

        def _upd_tail(g):
            # transpose (always PE: avoids the ~1.2us DMA completion-sem
            # latency and queue hops), logits add, softmax -> c16
            tp = ps_sm.tile([128, 512], f16, tag="sm", name=f"utp{it}_{g}")
            for k in range(4):
                nc.tensor.matmul(tp[:, 128 * k:128 * k + 128],
                                 u_sb[:, g, 128 * k:128 * k + 128], ident[:],
                                 is_transpose=True, start=(k == 0),
                                 stop=(k == 3), skip_group_check=True)
            _act_copy(nc, ut[:, g],
                      tp[:].rearrange("p (a b) -> p a b", b=128))
            srcu = ut[:, g].rearrange("p sc (b_ jp) -> p sc b_ jp", jp=32)
            nc.vector.tensor_add(
                logits[:, :, 4 * g:4 * g + 4, :],
                logits[:, :, 4 * g:4 * g + 4, :], srcu[:, :, :, 0:16])
            ex = small.tile([128, SC, 4, NCAP], f32, tag="ex")
            nc.scalar.activation(out=ex[:],
                                 in_=logits[:, :, 4 * g:4 * g + 4, :],
                                 func=EXP, scale=1.0, alpha=0.0)
            sm = small.tile([128, SC, 4, 1], f32, tag="sm")
            nc.vector.reduce_sum(sm[:], ex[:], axis=mybir.AxisListType.X)
            rc = small.tile([128, SC, 4, 1], f32, tag="rc")
            nc.vector.reciprocal(rc[:], sm[:])
            nc.vector.tensor_mul(c16[:, :, 4 * g:4 * g + 4, 0:16], ex[:],
                                 rc.broadcast_to([128, SC, 4, NCAP]))

        # hcq-outer so the p_sb-h1 transpose wait hides under g1's
        # hcq0 matmuls.  Evac for g0 is emitted right after its hcq1
        # block (ScalarE runs it under g1's matmuls); the PE-transpose
        # + softmax chains follow both blocks, with the g1 chain
        # hiding under the next iteration's y-g0.
        for hcq in range(2):
            for g in range(2):
                for hc in range(4 * hcq, 4 * hcq + 4):
                    for b_ in range(4):
                        b = 4 * g + b_
                        nc.tensor.matmul(
                            u_ps[g][32 * b_:32 * b_ + 16, :],
                            p_sb[:, hc, 16 * b:16 * b + 16],
                            xt16[:, b, hc, :],
                            start=False, stop=(hc == 7),
                            skip_group_check=True,
                            tile_position=(0, 32 * b_))
                if hcq == 1:
                    _act_copy(nc, u_sb[:, g, :], u_ps[g][:])
            if hcq == 0:
                # PE-transpose p-h1 here: pT_sb h1 evac finished during
                # the hcq0 matmuls, and hcq1 needs it immediately after
                tp1 = ps_sm.tile([128, 512], f16, tag="sm",
                                 name=f"ptp{it}")
                for k in range(4):
                    nc.tensor.matmul(tp1[:, 128 * k:128 * k + 128],
                                     pT_sb[:, 512 + 128 * k:640 + 128 * k],
                                     ident[:], is_transpose=True,
                                     start=(k == 0), stop=(k == 3),
                                     skip_group_check=True)
                _act_copy(nc, p_sb[:, 4:8, :],
                          tp1[:].rearrange("p (a b) -> p a b", b=128))
        for g in range(2):
            _upd_tail(g)
    ctx.close()


_CACHE = {}


def _host_consts():
    c0 = np.zeros((128, SC, BL, 32), np.float16)
    c0[:, :, :, 0:NCAP] = np.float16(1.0 / NCAP)
    logi = np.zeros((128, SC, BL, NCAP), np.float32)
    vblk0 = np.zeros((128, OC, 128), np.float16)
    ones2 = np.zeros((128, 2), np.float16)
    ones2[0:64, 0] = 1.0
    ones2[64:128, 1] = 1.0
    o2t = np.zeros((2, 128), np.float32)
    o2t[0, 0:64] = -1.0          # sign-fix for the single-Newton rsqrt
    o2t[1, 64:128] = -1.0
    zeros = np.zeros((128, 128), np.float16)
    magic = np.full((2, 1), MAGIC, np.int32)
    ident = np.eye(128, dtype=np.float16)
    return {"c0i": c0, "logi": logi, "vblki": vblk0, "ones2": ones2,
            "o2t": o2t, "zeros": zeros, "magic": magic, "ident": ident}


def _get_nc():
    if "nc" not in _CACHE:
        nc = bacc.Bacc("TRN2", target_bir_lowering=False, debug=False)
        x_d = nc.dram_tensor("x16", [128, BL, SC, 1024], f16,
                             kind="ExternalInput")
        xt_d = nc.dram_tensor("xt16", [128, BL, HC, 512], f16,
                              kind="ExternalInput")
        w_d = nc.dram_tensor("w16", [128, HC, 1024], f16,
                             kind="ExternalInput")
        wt_d = nc.dram_tensor("wt16", [128, OC, 1024], f16,
                              kind="ExternalInput")
        c0_d = nc.dram_tensor("c0i", [128, SC, BL, 32], f16,
                              kind="ExternalInput")
        logits_d = nc.dram_tensor("logi", [128, SC, BL, NCAP], f32,
                                  kind="ExternalInput")
        vblk_d = nc.dram_tensor("vblki", [128, OC, 128], f16,
                                kind="ExternalInput")
        ones2_d = nc.dram_tensor("ones2", [128, 2], f16, kind="ExternalInput")
        o2t_d = nc.dram_tensor("o2t", [2, 128], f32, kind="ExternalInput")
        zeros_d = nc.dram_tensor("zeros", [128, 128], f16,
                                 kind="ExternalInput")
        magic_d = nc.dram_tensor("magic", [2, 1], i32, kind="ExternalInput")
        ident_d = nc.dram_tensor("ident", [128, 128], f16,
                                 kind="ExternalInput")
        out_d = nc.dram_tensor("out", [128, 64], f32, kind="ExternalOutput")
        with tile.TileContext(nc) as tc:
            _build_kernel(tc, out_d.ap(), x_d.ap(), xt_d.ap(), w_d.ap(),
                          wt_d.ap(), c0_d.ap(), logits_d.ap(), vblk_d.ap(),
                          ones2_d.ap(), o2t_d.ap(), zeros_d.ap(),
                          magic_d.ap(), ident_d.ap())
        nc.compile()
        _CACHE["nc"] = nc
    return _CACHE["nc"]


def kernel(inputs: np.ndarray, W: np.ndarray, _trace: bool = False):
    """inputs: (512, 64, 1024) f32; W: (1, 1024, 1024) f32.
    Returns (64, 16, 64) f32."""
    nc = _get_nc()
    consts = _host_consts()
    w0 = W[0].astype(np.float16)
    w16h = np.ascontiguousarray(
        w0.reshape(HC, 128, 1024).transpose(1, 0, 2))
    wt16h = np.ascontiguousarray(
        w0.reshape(1024, OC, 128).transpose(2, 1, 0))
    x16f = inputs.astype(np.float16)              # (S, B, H)
    in_maps = []
    for c in range(N_CORES):
        xs = x16f[:, c * BL:(c + 1) * BL, :]      # (S, BL, H)
        x16h = np.ascontiguousarray(
            xs.reshape(SC, 128, BL, H).transpose(1, 2, 0, 3))
        xt16h = np.ascontiguousarray(
            xs.reshape(S, BL, HC, 128).transpose(3, 1, 2, 0))
        m = {"x16": x16h, "xt16": xt16h, "w16": w16h, "wt16": wt16h}
        m.update(consts)
        in_maps.append(m)
    kw = {}
    if _trace:
        kw = dict(trace=True, trace_cores=[0], stitch_traces=False)
    res = run_bass_kernel_spmd(nc, in_maps, core_ids=list(range(N_CORES)),
                               **kw)
    outs = []
    for c in range(N_CORES):
        v = res.results[c]["out"]          # (128=e*64+d, 64=b*8+oc)
        outs.append(v.reshape(2, 64, BL, 8).transpose(2, 3, 0, 1)
                     .reshape(BL, NCAP, DCAP))
    out = np.concatenate(outs, axis=0)
    if _trace:
        return out.astype(np.float32), res
    return out.astype(np.float32)


# revision 33
# speedup vs baseline: 1.4800x; 1.0048x over previous
"""Trainium2 Bass kernel for capsule dynamic routing (nn_Capsule) — v4.

Reference (per batch item b):
    u = x_b @ W; logits = 0
    for i in 4:
        c = softmax(logits, axis=capsule)
        t_j = sum_s c[s,j] * u[s, j*64:(j+1)*64]; v = squash(t)
        if i < 3: logits[s,j] += u[s, jblk] . v_j

Never materializes u (linearity):
    y_j   = sum_s c[s,j] x_s            y-GEMM   (c stationary, col-tiled)
    t     = W^T y^T                     t-GEMM   (w16 stationary per-slice)
    P^T   = Vblk^T W^T                  P-GEMM   (vblk stationary, block-diag)
    upd^T = P^T X                       upd-GEMM (P slices stationary, col-tiled)

v4 vs v3 (230us): trace showed PE active only 53%, HAM cold 40% of span.
  - queue discipline: ALL bulk input loads ride the scalar HWDGE queue;
    the sync queue carries only tiny consts then mid-iteration
    transposes.  (v3 split loads across both queues, so iteration-0/1
    transposes sat 40+us behind xt16 loads -> PE starved -> HAM cold.)
  - load order w16 -> x16 (per b,half chunks) -> wt16 -> xt16 (per
    b,hcq chunks), matching first-use order; iteration-0 y-GEMM is
    emitted batch-major so it consumes x16 chunks as they land.
  - per-stage emission reordered so every stage-bridging evac/
    transpose/softmax hides under another block's matmuls:
      y: half-outer (iters>=1) with per-(half,g) evac+transpose;
      upd: per-g softmax tail emitted between the two hcq1 g-blocks;
      t: per-oc block-diag extract emitted right after that oc's last
      accumulation matmul.
  - transposes alternate sync/scalar queues once loads are done.
  - warm fillers only where a real >3us PE gap is unavoidable
    (pre-stage), rhs=zeros so they never dep on input DMAs.

HW lessons kept from v3:
  - DVE copy PSUM(f32)->SBUF(f16) kills the device; PSUM->f16 casts go
    through ScalarE activation(Copy).
  - matmul start=True lazily zeroes the whole 2KB PSUM bank: accumulation
    groups must own a (partition-range x bank) region exclusively;
    partition-disjoint groups interleave with skip_group_check=True;
    column-disjoint writes into one bank are fine after the first
    start=True (has_written is per-element).
  - squash rsqrt on DVE (bitcast seed + 2 Newton steps); ScalarE runs
    only Copy+Exp -> exactly ONE ACT table load for the whole kernel.
  - nc.vector.memset on f16 tiles is unreliable: constants come from host.
"""
import numpy as np
from contextlib import ExitStack

import concourse.bass as bass
import concourse.bacc as bacc
import concourse.tile as tile
from concourse import mybir
from concourse.bass_utils import run_bass_kernel_spmd

f16 = mybir.dt.float16
f32 = mybir.dt.float32
i32 = mybir.dt.int32
COPY = mybir.ActivationFunctionType.Copy
EXP = mybir.ActivationFunctionType.Exp
MULT = mybir.AluOpType.mult
SUB = mybir.AluOpType.subtract
ADD = mybir.AluOpType.add
SHR = mybir.AluOpType.logical_shift_right

S, B, H = 512, 64, 1024
NCAP, DCAP = 16, 64
ROUTINGS = 4
N_CORES = 8
BL = B // N_CORES          # 8 batch items per core
SC = S // 128              # 4 s-chunks
HC = H // 128              # 8 h-chunks
OC = H // 128              # 8 o-chunks (o = NCAP*DCAP = 1024)
MAGIC = 0x5EF759DF         # rsqrt seed for h = s/2: 0x5f3759df - (1<<22)


def _act_copy(nc, out, in_):
    nc.scalar.activation(out=out, in_=in_, func=COPY, scale=1.0, alpha=0.0)


def _build_kernel(tc, out_d, x_d, xt_d, w_d, wt_d, c0_d, logits_d, vblk_d,
                  ones2_d, o2t_d, zeros_d, magic_d, ident_d):
    nc = tc.nc
    ctx = ExitStack()
    const = ctx.enter_context(tc.tile_pool(name="const", bufs=1))
    work = ctx.enter_context(tc.tile_pool(name="work", bufs=1))
    small = ctx.enter_context(tc.tile_pool(name="small", bufs=2))
    ps_big = ctx.enter_context(tc.tile_pool(name="ps_big", bufs=2,
                                            space="PSUM"))
    ps_u = ctx.enter_context(tc.tile_pool(name="ps_u", bufs=2, space="PSUM"))
    ps_sm = ctx.enter_context(tc.tile_pool(name="ps_sm", bufs=2,
                                           space="PSUM"))

    # ---------- persistent tensors ----------
    x16 = const.tile([128, BL, SC, 1024], f16)      # (s_loc, b, sc, h)
    xt16 = const.tile([128, BL, HC, 512], f16)      # (h_loc, b, hc, s)
    w16 = const.tile([128, HC, 1024], f16)          # (h_loc, hc, o)
    wt16 = const.tile([128, OC, 1024], f16)         # (o_loc, oc, h)
    c16 = const.tile([128, SC, BL, 32], f16)        # coeffs, cols 16-31 zero
    logits = const.tile([128, SC, BL, NCAP], f32)
    vblk = const.tile([128, OC, 128], f16)          # block-diag v, bj dense
    ones2 = const.tile([128, 2], f16)               # [[1;0],[0;1]] halves
    o2t = const.tile([2, 128], f32)                 # broadcast helper
    zeros = const.tile([128, 128], f16)             # zero-weight psum fill
    magic = const.tile([2, 1], i32)                 # rsqrt seed constant
    ident = const.tile([128, 128], f16)             # PE-transpose identity

    # ---------- loads ----------
    # ALL DMAs (loads + transposes) ride the sync (SP) queue: HWDGE DMA
    # issue blocks the issuing engine for ~0.5-1.5us per call, so the
    # scalar (ACT) engine must never issue DMAs or its evacs stall.
    # Loads are big per-b contiguous chunks in first-use order.
    # zeros+c16 first (warm fillers + y0 weights), then x b0-3 so the
    # iteration-0 y-GEMM starts ASAP; remaining consts ride behind.
    # w16 before x b4-7 (t-GEMM right after y); wt16 before xt16.
    nc.sync.dma_start(out=zeros[:], in_=zeros_d[:])
    nc.sync.dma_start(out=c16[:], in_=c0_d[:])
    for b in range(4):
        nc.sync.dma_start(out=x16[:, b], in_=x_d[:, b])
    nc.sync.dma_start(out=ident[:], in_=ident_d[:])
    nc.sync.dma_start(out=ones2[:], in_=ones2_d[:])
    nc.sync.dma_start(out=o2t[:], in_=o2t_d[:])
    nc.sync.dma_start(out=magic[:], in_=magic_d[:])
    nc.sync.dma_start(out=logits[:], in_=logits_d[:])
    nc.sync.dma_start(out=vblk[:], in_=vblk_d[:])
    for b in range(4, BL):
        nc.sync.dma_start(out=x16[:, b], in_=x_d[:, b])
    nc.sync.dma_start(out=w16[:], in_=w_d[:])
    nc.sync.dma_start(out=wt16[:], in_=wt_d[:])
    for b in range(BL):
        nc.sync.dma_start(out=xt16[:, b], in_=xt_d[:, b])

    def _warm(ps_tile, n, big=False):
        # zero-weight matmuls into a psum tile whose next real producer
        # begins with start=True (which wipes the bank): pure HAM fuel
        # that runs during the dependency-wait gap before the stage.
        # big=True streams x16 (N=512, 215ns each) for post-load fillers;
        # otherwise rhs=zeros so fillers never wait on input DMAs.
        for k in range(n):
            if big:
                nc.tensor.matmul(ps_tile[:, 0:512], zeros[:],
                                 x16[:, 0, 0, 0:512],
                                 start=(k == 0), stop=False,
                                 skip_group_check=True)
            else:
                nc.tensor.matmul(ps_tile[:, 0:128], zeros[:], zeros[:],
                                 start=(k == 0), stop=False,
                                 skip_group_check=True)

    def _tr(it, out3d, in2d, name):
        # SBUF transpose.  Iteration 0 uses PE transpose-mode (the HWDGE
        # transpose barrier + shared sem pool would serialize DMA
        # transposes against the in-flight input loads, stalling both);
        # PE is idle during the load window anyway and this keeps HAM
        # warm.  Iterations >=1 use xbar DMA transposes on sync.
        if it == 0:
            n = in2d.free_size() // 128
            tp = ps_sm.tile([128, 512], f16, tag="sm", name=name)
            for k in range(n):
                nc.tensor.matmul(tp[:, 128 * k:128 * k + 128],
                                 in2d[:, 128 * k:128 * k + 128], ident[:],
                                 is_transpose=True, start=(k == 0),
                                 stop=(k == n - 1), skip_group_check=True)
            _act_copy(nc, out3d,
                      tp[:, 0:128 * n].rearrange("p (a b) -> p a b", b=128))
        else:
            nc.sync.dma_start_transpose(out3d, in2d)

    for it in range(ROUTINGS):
        last = it == ROUTINGS - 1

        # ---------- y = C^T X ----------
        y_ps = [ps_big.tile([128, 1024], f32, tag="big", name=f"y{it}_{g}")
                for g in range(2)]
        if it == 0:
            _warm(y_ps[0], 16)
            import os
            if os.environ.get("DIAG_FILL"):
                _warm(y_ps[0], 300, big=True)
        else:
            # cover the tail-g1 softmax chain + keep HAM warm
            _warm(y_ps[0], 8, big=True)
        y_sb = work.tile([128, 2, 1024], f16, tag="y_sb")
        yt = work.tile([128, HC, 256], f16, tag="yt")

        def _y_mm(g, b_, half, sc):
            b = 4 * g + b_
            hs = slice(512 * half, 512 * half + 512)
            nc.tensor.matmul(
                y_ps[g][32 * b_:32 * b_ + 32, hs],
                c16[:, sc, b, :],
                x16[:, b, sc, hs],
                start=(sc == 0), stop=(sc == SC - 1),
                skip_group_check=True,
                tile_position=(0, 32 * b_))

        def _y_evac(g, half):
            hs = slice(512 * half, 512 * half + 512)
            _act_copy(nc, y_sb[:, g, hs], y_ps[g][:, hs])
            _tr(it, yt[:, 4 * half:4 * half + 4, 128 * g:128 * g + 128],
                y_sb[:, g, hs], f"ytr{it}_{half}_{g}")

        if it == 0:
            # batch-major: track x16 arrival order b0..b7
            for g in range(2):
                for b_ in range(4):
                    for half in range(2):
                        for sc in range(SC):
                            _y_mm(g, b_, half, sc)
                for half in range(2):
                    _y_evac(g, half)
        else:
            # half-outer: both h0 transposes complete during the h1
            # matmuls, so t-hcq0 starts right after y.  (Both groups'
            # softmax tails run in parallel across engines, so c16-g1
            # is ready in time.)
            for half in range(2):
                for g in range(2):
                    for b_ in range(4):
                        for sc in range(SC):
                            _y_mm(g, b_, half, sc)
                    _y_evac(g, half)

        # ---------- t = W^T y^T, two hc passes ----------
        yt_dense = yt.rearrange("p hc (g b_ jp) -> p hc g b_ jp", g=2, jp=32)
        t_ps = ps_big.tile([128, 1024], f32, tag="big", name=f"t{it}")
        _warm(t_ps, 2 if it == 0 else 6, big=(it > 0))
        for hcq in range(2):
            for oc in range(OC):
                for hc in range(4 * hcq, 4 * hcq + 4):
                    nc.tensor.matmul(
                        t_ps[:, oc * 128:oc * 128 + 128],
                        w16[:, hc, oc * 128:oc * 128 + 128],
                        yt_dense[:, hc, :, :, 0:16],
                        start=(hcq == 0 and hc == 0 and oc % 4 == 0),
                        stop=(hcq == 1 and hc == 7),
                        skip_group_check=True)


        # ---------- extract block-diag: t_sb (e*64+d, b*8+oc) ----------
        # t_ps col for (oc, b, j=2oc+e) = oc*130 + b*16 + e: linear in
        # (oc, b), so one strided copy per e-half (DVE + ScalarE in
        # parallel) replaces 16 tiny per-oc copies.
        t_sb = small.tile([128, 64], f32, tag="t_sb")
        pps = t_ps[:].ap[0][0]
        psb = t_sb[:].ap[0][0]
        for e in range(2):
            srcd = bass.AP(tensor=t_ps[:].tensor, offset=64 * e * pps + e,
                           ap=[[pps, 64], [130, OC], [16, BL]])
            dstd = bass.AP(tensor=t_sb[:].tensor, offset=64 * e * psb,
                           ap=[[psb, 64], [1, OC], [8, BL]])
            if e == 0:
                nc.vector.tensor_copy(dstd, srcd)
            else:
                _act_copy(nc, dstd, srcd)

        # ---------- squash: rs = rsqrt(sum_d t^2 + eps) on DVE ----------
        t2 = small.tile([128, 64], f16, tag="t2")
        nc.vector.tensor_mul(t2[:], t_sb[:], t_sb[:])
        sq_ps = ps_sm.tile([2, 512], f32, tag="sm", name=f"sq{it}")
        nc.tensor.matmul(sq_ps[:, 0:64], ones2[:], t2[:])
        h_sb = small.tile([2, 64], f32, tag="h_sb")
        nc.vector.tensor_scalar(out=h_sb[:], in0=sq_ps[:, 0:64],
                                scalar1=0.5, scalar2=5e-8, op0=MULT, op1=ADD)
        ri = small.tile([2, 64], i32, tag="ri")
        nc.vector.tensor_scalar(out=ri[:], in0=h_sb.bitcast(i32),
                                scalar1=1, scalar2=None, op0=SHR)
        r0 = small.tile([2, 64], f32, tag="r0")
        nc.vector.tensor_tensor(out=r0.bitcast(i32),
                                in0=magic.broadcast_to([2, 64]),
                                in1=ri[:], op=SUB)
        # Newton x1 with sign fold: rs = (h*r0*r0 - 1.5) * r0 = -rsqrt
        # approx (0.2% err, fine at 2e-2 tol); o2t carries -1 so the
        # broadcast flips the sign back.
        rr = small.tile([2, 64], f32, tag="rr")
        rs = small.tile([2, 64], f32, tag="rs")
        nc.vector.tensor_mul(rr[:], r0[:], r0[:])
        nc.vector.tensor_mul(rr[:], rr[:], h_sb[:])
        nc.vector.scalar_tensor_tensor(out=rs[:], in0=rr[:], scalar=1.5,
                                       in1=r0[:], op0=SUB, op1=MULT)
        # broadcast -rs (2,64) -> (128,64): bc[p,n] = -rs[p//64, n]
        bc_ps = ps_sm.tile([128, 512], f32, tag="sm", name=f"bc{it}")
        nc.tensor.matmul(bc_ps[:, 0:64], o2t[:], rs[:])

        if last:
            v32 = small.tile([128, 64], f32, tag="v32")
            nc.vector.tensor_mul(v32[:], t_sb[:], bc_ps[:, 0:64])
            # raw dump; host reorders (out[b,2oc+e,d] = v32[e*64+d, b*8+oc])
            nc.sync.dma_start(out=out_d, in_=v32[:])
            break

        # ---------- scatter v = t*bc into block-diag vblk (fused) -------
        # vblk[e*64+d, oc, b*16+2oc+e] = t_sb[e*64+d, b*8+oc] * bc[...]
        pitch = vblk[:].ap[0][0]
        for e in range(2):
            dst = bass.AP(tensor=vblk[:].tensor,
                          offset=64 * e * pitch + e,
                          ap=[[pitch, 64], [130, OC], [16, BL]])
            nc.vector.tensor_tensor(
                out=dst,
                in0=t_sb[64 * e:64 * e + 64, :].rearrange(
                    "p (b o) -> p o b", o=OC),
                in1=bc_ps[64 * e:64 * e + 64, 0:64].rearrange(
                    "p (b o) -> p o b", o=OC),
                op=MULT)

        # ---------- P^T = Vblk^T W^T, h-halves pipelined ----------
        pT_ps = ps_big.tile([128, 1024], f32, tag="big", name=f"pT{it}")
        _warm(pT_ps, 3 if it == 0 else 8, big=(it > 0))
        pT_sb = work.tile([128, 1024], f16, tag="pT_sb")
        p_sb = work.tile([128, HC, 128], f16, tag="p_sb")
        for half in range(2):
            hs = slice(512 * half, 512 * half + 512)
            for oc in range(OC):
                nc.tensor.matmul(
                    pT_ps[:, hs],
                    vblk[:, oc, :],
                    wt16[:, oc, hs],
                    start=(oc == 0), stop=(oc == OC - 1),
                    skip_group_check=True)
            _act_copy(nc, pT_sb[:, hs], pT_ps[:, hs])
            if half == 0:
                # h0 rides the xbar DMA (overlaps the h1 matmuls); h1
                # goes PE-transpose, emitted between the upd hcq blocks
                _tr(it, p_sb[:, 0:4, :], pT_sb[:, hs], f"ptr{it}_0")

        # ---------- upd^T = P^T X (col-tiled per batch) ----------
        # 2 groups of 4 batches (4-way col-tiled); each group's softmax
        # tail chain hides under the other group's matmuls / next y-g0.
        u_ps = [ps_u.tile([128, 512], f32, tag="u", name=f"u{it}_{g}")
                for g in range(2)]
        u_sb = work.tile([128, 2, 512], f16, tag="u_sb")
        ut = work.tile([128, 2, SC, 128], f16, tag="ut")
        _warm(u_ps[0], 3 if it == 0 else 6, big=(it > 0))
        for g in range(2):
            nc.tensor.matmul(u_ps[g][:], zeros[:], x16[:, 0, 0, 0:512],
                             start=True, stop=False, skip_group_check=True)

        def _upd_tail(g):
            # transpose (always PE: avoids the ~1.2us DMA completion-sem
            # latency and queue hops), logits add, softmax -> c16
            tp = ps_sm.tile([128, 512], f16, tag="sm", name=f"utp{it}_{g}")
            for k in range(4):
                nc.tensor.matmul(tp[:, 128 * k:128 * k + 128],
                                 u_sb[:, g, 128 * k:128 * k + 128], ident[:],
                                 is_transpose=True, start=(k == 0),
                                 stop=(k == 3), skip_group_check=True)
            _act_copy(nc, ut[:, g],
                      tp[:].rearrange("p (a b) -> p a b", b=128))
            srcu = ut[:, g].rearrange("p sc (b_ jp) -> p sc b_ jp", jp=32)
            nc.vector.tensor_add(
                logits[:, :, 4 * g:4 * g + 4, :],
                logits[:, :, 4 * g:4 * g + 4, :], srcu[:, :, :, 0:16])
            ex = small.tile([128, SC, 4, NCAP], f32, tag="ex")
            nc.scalar.activation(out=ex[:],
                                 in_=logits[:, :, 4 * g:4 * g + 4, :],
                                 func=EXP, scale=1.0, alpha=0.0)
            sm = small.tile([128, SC, 4, 1], f32, tag="sm")
            nc.vector.reduce_sum(sm[:], ex[:], axis=mybir.AxisListType.X)
            rc = small.tile([128, SC, 4, 1], f32, tag="rc")
            nc.vector.reciprocal(rc[:], sm[:])
            nc.vector.tensor_mul(c16[:, :, 4 * g:4 * g + 4, 0:16], ex[:],
                                 rc.broadcast_to([128, SC, 4, NCAP]))

        # hcq-outer so the p_sb-h1 transpose wait hides under g1's
        # hcq0 matmuls.  Evac for g0 is emitted right after its hcq1
        # block (ScalarE runs it under g1's matmuls); the PE-transpose
        # + softmax chains follow both blocks, with the g1 chain
        # hiding under the next iteration's y-g0.
        for hcq in range(2):
            for g in range(2):
                for hc in range(4 * hcq, 4 * hcq + 4):
                    for b_ in range(4):
                        b = 4 * g + b_
                        nc.tensor.matmul(
                            u_ps[g][32 * b_:32 * b_ + 16, :],
                            p_sb[:, hc, 16 * b:16 * b + 16],
                            xt16[:, b, hc, :],
                            start=False, stop=(hc == 7),
                            skip_group_check=True,
                            tile_position=(0, 32 * b_))
                if hcq == 1:
                    _act_copy(nc, u_sb[:, g, :], u_ps[g][:])
                if hcq == 0 and g == 0:
                    # PE-transpose p-h1 between the hcq0 g-blocks: its
                    # ACT evac lands during g1's matmuls, so hcq1
                    # starts with no stall
                    tp1 = ps_sm.tile([128, 512], f16, tag="sm",
                                     name=f"ptp{it}")
                    for k in range(4):
                        nc.tensor.matmul(
                            tp1[:, 128 * k:128 * k + 128],
                            pT_sb[:, 512 + 128 * k:640 + 128 * k],
                            ident[:], is_transpose=True,
                            start=(k == 0), stop=(k == 3),
                            skip_group_check=True)
                    _act_copy(nc, p_sb[:, 4:8, :],
                              tp1[:].rearrange("p (a b) -> p a b", b=128))
        for g in range(2):
            _upd_tail(g)
    ctx.close()


_CACHE = {}


def _host_consts():
    c0 = np.zeros((128, SC, BL, 32), np.float16)
    c0[:, :, :, 0:NCAP] = np.float16(1.0 / NCAP)
    logi = np.zeros((128, SC, BL, NCAP), np.float32)
    vblk0 = np.zeros((128, OC, 128), np.float16)
    ones2 = np.zeros((128, 2), np.float16)
    ones2[0:64, 0] = 1.0
    ones2[64:128, 1] = 1.0
    o2t = np.zeros((2, 128), np.float32)
    o2t[0, 0:64] = -1.0          # sign-fix for the single-Newton rsqrt
    o2t[1, 64:128] = -1.0
    zeros = np.zeros((128, 128), np.float16)
    magic = np.full((2, 1), MAGIC, np.int32)
    ident = np.eye(128, dtype=np.float16)
    return {"c0i": c0, "logi": logi, "vblki": vblk0, "ones2": ones2,
            "o2t": o2t, "zeros": zeros, "magic": magic, "ident": ident}


def _get_nc():
    if "nc" not in _CACHE:
        nc = bacc.Bacc("TRN2", target_bir_lowering=False, debug=False)
        x_d = nc.dram_tensor("x16", [128, BL, SC, 1024], f16,
                             kind="ExternalInput")
        xt_d = nc.dram_tensor("xt16", [128, BL, HC, 512], f16,
                              kind="ExternalInput")
        w_d = nc.dram_tensor("w16", [128, HC, 1024], f16,
                             kind="ExternalInput")
        wt_d = nc.dram_tensor("wt16", [128, OC, 1024], f16,
                              kind="ExternalInput")
        c0_d = nc.dram_tensor("c0i", [128, SC, BL, 32], f16,
                              kind="ExternalInput")
        logits_d = nc.dram_tensor("logi", [128, SC, BL, NCAP], f32,
                                  kind="ExternalInput")
        vblk_d = nc.dram_tensor("vblki", [128, OC, 128], f16,
                                kind="ExternalInput")
        ones2_d = nc.dram_tensor("ones2", [128, 2], f16, kind="ExternalInput")
        o2t_d = nc.dram_tensor("o2t", [2, 128], f32, kind="ExternalInput")
        zeros_d = nc.dram_tensor("zeros", [128, 128], f16,
                                 kind="ExternalInput")
        magic_d = nc.dram_tensor("magic", [2, 1], i32, kind="ExternalInput")
        ident_d = nc.dram_tensor("ident", [128, 128], f16,
                                 kind="ExternalInput")
        out_d = nc.dram_tensor("out", [128, 64], f32, kind="ExternalOutput")
        with tile.TileContext(nc) as tc:
            _build_kernel(tc, out_d.ap(), x_d.ap(), xt_d.ap(), w_d.ap(),
                          wt_d.ap(), c0_d.ap(), logits_d.ap(), vblk_d.ap(),
                          ones2_d.ap(), o2t_d.ap(), zeros_d.ap(),
                          magic_d.ap(), ident_d.ap())
        nc.compile()
        _CACHE["nc"] = nc
    return _CACHE["nc"]


def kernel(inputs: np.ndarray, W: np.ndarray, _trace: bool = False):
    """inputs: (512, 64, 1024) f32; W: (1, 1024, 1024) f32.
    Returns (64, 16, 64) f32."""
    nc = _get_nc()
    consts = _host_consts()
    w0 = W[0].astype(np.float16)
    w16h = np.ascontiguousarray(
        w0.reshape(HC, 128, 1024).transpose(1, 0, 2))
    wt16h = np.ascontiguousarray(
        w0.reshape(1024, OC, 128).transpose(2, 1, 0))
    x16f = inputs.astype(np.float16)              # (S, B, H)
    in_maps = []
    for c in range(N_CORES):
        xs = x16f[:, c * BL:(c + 1) * BL, :]      # (S, BL, H)
        x16h = np.ascontiguousarray(
            xs.reshape(SC, 128, BL, H).transpose(1, 2, 0, 3))
        xt16h = np.ascontiguousarray(
            xs.reshape(S, BL, HC, 128).transpose(3, 1, 2, 0))
        m = {"x16": x16h, "xt16": xt16h, "w16": w16h, "wt16": wt16h}
        m.update(consts)
        in_maps.append(m)
    kw = {}
    if _trace:
        kw = dict(trace=True, trace_cores=[0], stitch_traces=False)
    res = run_bass_kernel_spmd(nc, in_maps, core_ids=list(range(N_CORES)),
                               **kw)
    outs = []
    for c in range(N_CORES):
        v = res.results[c]["out"]          # (128=e*64+d, 64=b*8+oc)
        outs.append(v.reshape(2, 64, BL, 8).transpose(2, 3, 0, 1)
                     .reshape(BL, NCAP, DCAP))
    out = np.concatenate(outs, axis=0)
    if _trace:
        return out.astype(np.float32), res
    return out.astype(np.float32)


# revision 38
# speedup vs baseline: 1.5447x; 1.0437x over previous
"""Trainium2 Bass kernel for capsule dynamic routing (nn_Capsule) — v4.

Reference (per batch item b):
    u = x_b @ W; logits = 0
    for i in 4:
        c = softmax(logits, axis=capsule)
        t_j = sum_s c[s,j] * u[s, j*64:(j+1)*64]; v = squash(t)
        if i < 3: logits[s,j] += u[s, jblk] . v_j

Never materializes u (linearity):
    y_j   = sum_s c[s,j] x_s            y-GEMM   (c stationary, col-tiled)
    t     = W^T y^T                     t-GEMM   (w16 stationary per-slice)
    P^T   = Vblk^T W^T                  P-GEMM   (vblk stationary, block-diag)
    upd^T = P^T X                       upd-GEMM (P slices stationary, col-tiled)

v4 vs v3 (230us): trace showed PE active only 53%, HAM cold 40% of span.
  - queue discipline: ALL bulk input loads ride the scalar HWDGE queue;
    the sync queue carries only tiny consts then mid-iteration
    transposes.  (v3 split loads across both queues, so iteration-0/1
    transposes sat 40+us behind xt16 loads -> PE starved -> HAM cold.)
  - load order w16 -> x16 (per b,half chunks) -> wt16 -> xt16 (per
    b,hcq chunks), matching first-use order; iteration-0 y-GEMM is
    emitted batch-major so it consumes x16 chunks as they land.
  - per-stage emission reordered so every stage-bridging evac/
    transpose/softmax hides under another block's matmuls:
      y: half-outer (iters>=1) with per-(half,g) evac+transpose;
      upd: per-g softmax tail emitted between the two hcq1 g-blocks;
      t: per-oc block-diag extract emitted right after that oc's last
      accumulation matmul.
  - transposes alternate sync/scalar queues once loads are done.
  - warm fillers only where a real >3us PE gap is unavoidable
    (pre-stage), rhs=zeros so they never dep on input DMAs.

HW lessons kept from v3:
  - DVE copy PSUM(f32)->SBUF(f16) kills the device; PSUM->f16 casts go
    through ScalarE activation(Copy).
  - matmul start=True lazily zeroes the whole 2KB PSUM bank: accumulation
    groups must own a (partition-range x bank) region exclusively;
    partition-disjoint groups interleave with skip_group_check=True;
    column-disjoint writes into one bank are fine after the first
    start=True (has_written is per-element).
  - squash rsqrt on DVE (bitcast seed + 2 Newton steps); ScalarE runs
    only Copy+Exp -> exactly ONE ACT table load for the whole kernel.
  - nc.vector.memset on f16 tiles is unreliable: constants come from host.
"""
import numpy as np
from contextlib import ExitStack

import concourse.bass as bass
import concourse.bacc as bacc
import concourse.tile as tile
from concourse import mybir
from concourse.bass_utils import run_bass_kernel_spmd

f16 = mybir.dt.float16
f32 = mybir.dt.float32
i32 = mybir.dt.int32
COPY = mybir.ActivationFunctionType.Copy
EXP = mybir.ActivationFunctionType.Exp
MULT = mybir.AluOpType.mult
SUB = mybir.AluOpType.subtract
ADD = mybir.AluOpType.add
SHR = mybir.AluOpType.logical_shift_right

S, B, H = 512, 64, 1024
NCAP, DCAP = 16, 64
ROUTINGS = 4
N_CORES = 8
BL = B // N_CORES          # 8 batch items per core
SC = S // 128              # 4 s-chunks
HC = H // 128              # 8 h-chunks
OC = H // 128              # 8 o-chunks (o = NCAP*DCAP = 1024)
MAGIC = 0x5EF759DF         # rsqrt seed for h = s/2: 0x5f3759df - (1<<22)


def _act_copy(nc, out, in_):
    nc.scalar.activation(out=out, in_=in_, func=COPY, scale=1.0, alpha=0.0)


def _build_kernel(tc, out_d, x_d, xt_d, w_d, wt_d, c0_d, logits_d, vblk_d,
                  ones2_d, o2t_d, zeros_d, magic_d, ident_d, magic64_d):
    nc = tc.nc
    ctx = ExitStack()
    const = ctx.enter_context(tc.tile_pool(name="const", bufs=1))
    work = ctx.enter_context(tc.tile_pool(name="work", bufs=1))
    small = ctx.enter_context(tc.tile_pool(name="small", bufs=2))
    ps_big = ctx.enter_context(tc.tile_pool(name="ps_big", bufs=2,
                                            space="PSUM"))
    ps_u = ctx.enter_context(tc.tile_pool(name="ps_u", bufs=2, space="PSUM"))
    ps_sm = ctx.enter_context(tc.tile_pool(name="ps_sm", bufs=2,
                                           space="PSUM"))

    # ---------- persistent tensors ----------
    x16 = const.tile([128, BL, SC, 1024], f16)      # (s_loc, b, sc, h)
    xt16 = const.tile([128, BL, HC, 512], f16)      # (h_loc, b, hc, s)
    w16 = const.tile([128, HC, 1024], f16)          # (h_loc, hc, o)
    wt16 = const.tile([128, OC, 1024], f16)         # (o_loc, oc, h)
    c16 = const.tile([128, SC, BL, 32], f16)        # coeffs, cols 16-31 zero
    logits = const.tile([128, SC, BL, NCAP], f32)
    vblk = const.tile([128, OC, 128], f16)          # block-diag v, bj dense
    ones2 = const.tile([128, 2], f16)               # [[1;0],[0;1]] halves
    o2t = const.tile([2, 128], f32)                 # broadcast helper
    zeros = const.tile([128, 128], f16)             # zero-weight psum fill
    magic = const.tile([2, 1], i32)                 # rsqrt seed constant
    magic64 = const.tile([64, 1], i32)              # seed, [64,2] layout
    ident = const.tile([128, 128], f16)             # PE-transpose identity

    # ---------- loads ----------
    # ALL DMAs (loads + transposes) ride the sync (SP) queue: HWDGE DMA
    # issue blocks the issuing engine for ~0.5-1.5us per call, so the
    # scalar (ACT) engine must never issue DMAs or its evacs stall.
    # Loads are big per-b contiguous chunks in first-use order.
    # zeros+c16 first (warm fillers + y0 weights), then x b0-3 so the
    # iteration-0 y-GEMM starts ASAP; remaining consts ride behind.
    # w16 before x b4-7 (t-GEMM right after y); wt16 before xt16.
    nc.sync.dma_start(out=zeros[:], in_=zeros_d[:])
    nc.sync.dma_start(out=c16[:], in_=c0_d[:])
    for b in range(4):
        nc.sync.dma_start(out=x16[:, b], in_=x_d[:, b])
    nc.sync.dma_start(out=ident[:], in_=ident_d[:])
    nc.sync.dma_start(out=ones2[:], in_=ones2_d[:])
    nc.sync.dma_start(out=o2t[:], in_=o2t_d[:])
    nc.sync.dma_start(out=magic[:], in_=magic_d[:])
    nc.sync.dma_start(out=magic64[:], in_=magic64_d[:])
    nc.sync.dma_start(out=logits[:], in_=logits_d[:])
    nc.sync.dma_start(out=vblk[:], in_=vblk_d[:])
    for b in range(4, BL):
        nc.sync.dma_start(out=x16[:, b], in_=x_d[:, b])
    nc.sync.dma_start(out=w16[:], in_=w_d[:])
    nc.sync.dma_start(out=wt16[:], in_=wt_d[:])
    for b in range(BL):
        nc.sync.dma_start(out=xt16[:, b], in_=xt_d[:, b])

    def _warm(ps_tile, n, big=False):
        # zero-weight matmuls into a psum tile whose next real producer
        # begins with start=True (which wipes the bank): pure HAM fuel
        # that runs during the dependency-wait gap before the stage.
        # big=True streams x16 (N=512, 215ns each) for post-load fillers;
        # otherwise rhs=zeros so fillers never wait on input DMAs.
        for k in range(n):
            if big:
                nc.tensor.matmul(ps_tile[:, 0:512], zeros[:],
                                 x16[:, 0, 0, 0:512],
                                 start=(k == 0), stop=False,
                                 skip_group_check=True)
            else:
                nc.tensor.matmul(ps_tile[:, 0:128], zeros[:], zeros[:],
                                 start=(k == 0), stop=False,
                                 skip_group_check=True)

    def _tr(it, out3d, in2d, name):
        # SBUF transpose.  Iteration 0 uses PE transpose-mode (the HWDGE
        # transpose barrier + shared sem pool would serialize DMA
        # transposes against the in-flight input loads, stalling both);
        # PE is idle during the load window anyway and this keeps HAM
        # warm.  Iterations >=1 use xbar DMA transposes on sync.
        if it == 0:
            n = in2d.free_size() // 128
            tp = ps_sm.tile([128, 512], f16, tag="sm", name=name)
            for k in range(n):
                nc.tensor.matmul(tp[:, 128 * k:128 * k + 128],
                                 in2d[:, 128 * k:128 * k + 128], ident[:],
                                 is_transpose=True, start=(k == 0),
                                 stop=(k == n - 1), skip_group_check=True)
            _act_copy(nc, out3d,
                      tp[:, 0:128 * n].rearrange("p (a b) -> p a b", b=128))
        else:
            nc.sync.dma_start_transpose(out3d, in2d)

    for it in range(ROUTINGS):
        last = it == ROUTINGS - 1

        # ---------- y = C^T X ----------
        y_ps = [ps_big.tile([128, 1024], f32, tag="big", name=f"y{it}_{g}")
                for g in range(2)]
        if it == 0:
            _warm(y_ps[0], 16)
            import os
            if os.environ.get("DIAG_FILL"):
                _warm(y_ps[0], 300, big=True)
        else:
            # cover the tail-g1 softmax chain + keep HAM warm
            _warm(y_ps[0], 8, big=True)
        y_sb = work.tile([128, 2, 1024], f16, tag="y_sb")
        yt = work.tile([128, HC, 256], f16, tag="yt")

        def _y_mm(g, b_, half, sc):
            b = 4 * g + b_
            hs = slice(512 * half, 512 * half + 512)
            nc.tensor.matmul(
                y_ps[g][32 * b_:32 * b_ + 32, hs],
                c16[:, sc, b, :],
                x16[:, b, sc, hs],
                start=(sc == 0), stop=(sc == SC - 1),
                skip_group_check=True,
                tile_position=(0, 32 * b_))

        def _y_evac(g, half):
            hs = slice(512 * half, 512 * half + 512)
            _act_copy(nc, y_sb[:, g, hs], y_ps[g][:, hs])
            _tr(it, yt[:, 4 * half:4 * half + 4, 128 * g:128 * g + 128],
                y_sb[:, g, hs], f"ytr{it}_{half}_{g}")

        if it == 0:
            # batch-major: track x16 arrival order b0..b7
            for g in range(2):
                for b_ in range(4):
                    for half in range(2):
                        for sc in range(SC):
                            _y_mm(g, b_, half, sc)
                for half in range(2):
                    _y_evac(g, half)
        else:
            # half-outer: both h0 transposes complete during the h1
            # matmuls, so t-hcq0 starts right after y.  (Both groups'
            # softmax tails run in parallel across engines, so c16-g1
            # is ready in time.)
            for half in range(2):
                for g in range(2):
                    for b_ in range(4):
                        for sc in range(SC):
                            _y_mm(g, b_, half, sc)
                    _y_evac(g, half)

        # ---------- t = W^T y^T, two hc passes ----------
        yt_dense = yt.rearrange("p hc (g b_ jp) -> p hc g b_ jp", g=2, jp=32)
        t_ps = ps_big.tile([128, 1024], f32, tag="big", name=f"t{it}")
        _warm(t_ps, 2 if it == 0 else 6, big=(it > 0))
        for hcq in range(2):
            for oc in range(OC):
                for hc in range(4 * hcq, 4 * hcq + 4):
                    nc.tensor.matmul(
                        t_ps[:, oc * 128:oc * 128 + 128],
                        w16[:, hc, oc * 128:oc * 128 + 128],
                        yt_dense[:, hc, :, :, 0:16],
                        start=(hcq == 0 and hc == 0 and oc % 4 == 0),
                        stop=(hcq == 1 and hc == 7),
                        skip_group_check=True)


        # ---------- extract block-diag: t_sb (e*64+d, b*8+oc) ----------
        # t_ps col for (oc, b, j=2oc+e) = oc*130 + b*16 + e: linear in
        # (oc, b), so one strided copy per e-half (DVE + ScalarE in
        # parallel) replaces 16 tiny per-oc copies.
        t_sb = small.tile([128, 64], f32, tag="t_sb")
        pps = t_ps[:].ap[0][0]
        psb = t_sb[:].ap[0][0]
        for e in range(2):
            # yt/c16 columns carry capsules in j' = 8e+oc order (see the
            # vblk packing below), so the diagonal for (b, oc, e) sits at
            # flat col 129*oc + 16*b + 8*e
            srcd = bass.AP(tensor=t_ps[:].tensor,
                           offset=64 * e * pps + 8 * e,
                           ap=[[pps, 64], [129, OC], [16, BL]])
            dstd = bass.AP(tensor=t_sb[:].tensor, offset=64 * e * psb,
                           ap=[[psb, 64], [1, OC], [8, BL]])
            if e == 0:
                nc.vector.tensor_copy(dstd, srcd)
            else:
                _act_copy(nc, dstd, srcd)

        # ---------- scatter unnormalized t into block-diag vblk ----------
        # vblk[64e+d, oc, 64e+8b+oc] = t_sb[64e+d, 8b+oc]; column packing
        # 64e+8b+oc makes the j' = 8e+oc capsule order used everywhere
        # downstream (softmax over j is order-agnostic)
        if not last:
            pitch = vblk[:].ap[0][0]
            for e in range(2):
                dst = bass.AP(tensor=vblk[:].tensor,
                              offset=64 * e * pitch + 64 * e,
                              ap=[[pitch, 64], [129, OC], [8, BL]])
                nc.vector.tensor_copy(
                    dst,
                    t_sb[64 * e:64 * e + 64, :].rearrange(
                        "p (b o) -> p o b", o=OC))

        # ---------- squash: rs = rsqrt(sum_d t^2 + eps) on DVE ----------
        t2 = small.tile([128, 64], f16, tag="t2")
        nc.vector.tensor_mul(t2[:], t_sb[:], t_sb[:])

        if last:
            # [2,64] layout: partition-sum via ones2, broadcast via o2t
            sq_ps = ps_sm.tile([2, 512], f32, tag="sm", name=f"sq{it}")
            nc.tensor.matmul(sq_ps[:, 0:64], ones2[:], t2[:])
            h_sb = small.tile([2, 64], f32, tag="h_sb")
            nc.vector.tensor_scalar(out=h_sb[:], in0=sq_ps[:, 0:64],
                                    scalar1=0.5, scalar2=5e-8,
                                    op0=MULT, op1=ADD)
            ri = small.tile([2, 64], i32, tag="ri")
            nc.vector.tensor_scalar(out=ri[:], in0=h_sb.bitcast(i32),
                                    scalar1=1, scalar2=None, op0=SHR)
            r0 = small.tile([2, 64], f32, tag="r0")
            nc.vector.tensor_tensor(out=r0.bitcast(i32),
                                    in0=magic.broadcast_to([2, 64]),
                                    in1=ri[:], op=SUB)
            # Newton x1, sign-folded negative; o2t carries -1 to flip
            rr = small.tile([2, 64], f32, tag="rr")
            rs = small.tile([2, 64], f32, tag="rs")
            nc.vector.tensor_mul(rr[:], r0[:], r0[:])
            nc.vector.tensor_mul(rr[:], rr[:], h_sb[:])
            nc.vector.scalar_tensor_tensor(out=rs[:], in0=rr[:], scalar=1.5,
                                           in1=r0[:], op0=SUB, op1=MULT)
            bc_ps = ps_sm.tile([128, 512], f32, tag="sm", name=f"bc{it}")
            nc.tensor.matmul(bc_ps[:, 0:64], o2t[:], rs[:])
            v32 = small.tile([128, 64], f32, tag="v32")
            nc.vector.tensor_mul(v32[:], t_sb[:], bc_ps[:, 0:64])
            # raw dump; host reorders (out[b,2oc+e,d] = v32[e*64+d, b*8+oc])
            nc.sync.dma_start(out=out_d, in_=v32[:])
            break

        # [64,2] layout: sums64[m, e] = sum_d t2[64e+d, m] in ONE matmul
        # (lhsT = t2, rhs = ones2); Newton runs there, then two
        # partition-offset copies (with sign flip) build the per-pT-row
        # scale rs_bj[64e+m] = rsqrt (applied at the pT evac; linearity:
        # upd = rs * (X W^T t), so P runs on unnormalized t).
        sq64 = ps_sm.tile([128, 512], f32, tag="sm", name=f"sq{it}")
        nc.tensor.matmul(sq64[0:64, 0:2], t2[:], ones2[:])
        h64 = small.tile([64, 2], f32, tag="h_sb")
        nc.vector.tensor_scalar(out=h64[:], in0=sq64[0:64, 0:2],
                                scalar1=0.5, scalar2=5e-8,
                                op0=MULT, op1=ADD)
        ri64 = small.tile([64, 2], i32, tag="ri")
        nc.vector.tensor_scalar(out=ri64[:], in0=h64.bitcast(i32),
                                scalar1=1, scalar2=None, op0=SHR)
        r064 = small.tile([64, 2], f32, tag="r0")
        nc.vector.tensor_tensor(out=r064.bitcast(i32),
                                in0=magic64.broadcast_to([64, 2]),
                                in1=ri64[:], op=SUB)
        rr64 = small.tile([64, 2], f32, tag="rr")
        rs64 = small.tile([64, 2], f32, tag="rs")
        nc.vector.tensor_mul(rr64[:], r064[:], r064[:])
        nc.vector.tensor_mul(rr64[:], rr64[:], h64[:])
        nc.vector.scalar_tensor_tensor(out=rs64[:], in0=rr64[:], scalar=1.5,
                                       in1=r064[:], op0=SUB, op1=MULT)
        rs_bj = small.tile([128, 1], f32, tag="rs_bj")
        for e in range(2):
            nc.vector.tensor_scalar(out=rs_bj[64 * e:64 * e + 64, :],
                                    in0=rs64[:, e:e + 1], scalar1=-1.0,
                                    scalar2=None, op0=MULT)

        # ---------- P^T = Vblk^T W^T, h-halves pipelined ----------
        pT_ps = ps_big.tile([128, 1024], f32, tag="big", name=f"pT{it}")
        _warm(pT_ps, 3 if it == 0 else 8, big=(it > 0))
        pT_sb = work.tile([128, 1024], f16, tag="pT_sb")
        p_sb = work.tile([128, HC, 128], f16, tag="p_sb")
        for half in range(2):
            hs = slice(512 * half, 512 * half + 512)
            for oc in range(OC):
                nc.tensor.matmul(
                    pT_ps[:, hs],
                    vblk[:, oc, :],
                    wt16[:, oc, hs],
                    start=(oc == 0), stop=(oc == OC - 1),
                    skip_group_check=True)
            nc.scalar.activation(out=pT_sb[:, hs], in_=pT_ps[:, hs],
                                 func=COPY, scale=rs_bj[:], alpha=0.0)

        def _p_tr(half):
            # PE-transpose pT half -> tp, then ACT evac with a permuting
            # AP: tp col (hc, 64e+8b+oc) lands at p_sb col 16b+8e+oc so
            # upd's stationary slice p_sb[:, hc, 16b:16b+16] stays a
            # single contiguous free dim.
            hs = slice(512 * half, 512 * half + 512)
            tp = ps_sm.tile([128, 512], f16, tag="sm", name=f"ptp{it}_{half}")
            for k in range(4):
                nc.tensor.matmul(tp[:, 128 * k:128 * k + 128],
                                 pT_sb[:, 512 * half + 128 * k:
                                       512 * half + 128 * k + 128],
                                 ident[:], is_transpose=True,
                                 start=(k == 0), stop=(k == 3),
                                 skip_group_check=True)
            ppsb = p_sb[:].ap[0][0]
            tpv = tp[:].rearrange("p (k e b q) -> p k e b q", k=4, e=2, b=8)
            for e in range(2):
                dst = bass.AP(tensor=p_sb[:].tensor,
                              offset=512 * half + 8 * e,
                              ap=[[ppsb, 128], [128, 4], [16, 8], [1, 8]])
                _act_copy(nc, dst, tpv[:, :, e, :, :])

        _p_tr(0)

        # ---------- upd^T = P^T X (col-tiled per batch) ----------
        # 2 groups of 4 batches (4-way col-tiled); each group's softmax
        # tail chain hides under the other group's matmuls / next y-g0.
        u_ps = [ps_u.tile([128, 512], f32, tag="u", name=f"u{it}_{g}")
                for g in range(2)]
        u_sb = work.tile([128, 2, 512], f16, tag="u_sb")
        ut = work.tile([128, 2, SC, 128], f16, tag="ut")
        _warm(u_ps[0], 3 if it == 0 else 6, big=(it > 0))
        for g in range(2):
            nc.tensor.matmul(u_ps[g][:], zeros[:], x16[:, 0, 0, 0:512],
                             start=True, stop=False, skip_group_check=True)

        def _upd_tail(g):
            # transpose (always PE: avoids the ~1.2us DMA completion-sem
            # latency and queue hops), logits add, softmax -> c16
            tp = ps_sm.tile([128, 512], f16, tag="sm", name=f"utp{it}_{g}")
            for k in range(4):
                nc.tensor.matmul(tp[:, 128 * k:128 * k + 128],
                                 u_sb[:, g, 128 * k:128 * k + 128], ident[:],
                                 is_transpose=True, start=(k == 0),
                                 stop=(k == 3), skip_group_check=True)
            _act_copy(nc, ut[:, g],
                      tp[:].rearrange("p (a b) -> p a b", b=128))
            srcu = ut[:, g].rearrange("p sc (b_ jp) -> p sc b_ jp", jp=32)
            nc.vector.tensor_add(
                logits[:, :, 4 * g:4 * g + 4, :],
                logits[:, :, 4 * g:4 * g + 4, :], srcu[:, :, :, 0:16])
            ex = small.tile([128, SC, 4, NCAP], f32, tag="ex")
            nc.scalar.activation(out=ex[:],
                                 in_=logits[:, :, 4 * g:4 * g + 4, :],
                                 func=EXP, scale=1.0, alpha=0.0)
            sm = small.tile([128, SC, 4, 1], f32, tag="sm")
            nc.vector.reduce_sum(sm[:], ex[:], axis=mybir.AxisListType.X)
            rc = small.tile([128, SC, 4, 1], f32, tag="rc")
            nc.vector.reciprocal(rc[:], sm[:])
            nc.vector.tensor_mul(c16[:, :, 4 * g:4 * g + 4, 0:16], ex[:],
                                 rc.broadcast_to([128, SC, 4, NCAP]))

        # hcq-outer so the p_sb-h1 transpose wait hides under g1's
        # hcq0 matmuls.  Evac for g0 is emitted right after its hcq1
        # block (ScalarE runs it under g1's matmuls); the PE-transpose
        # + softmax chains follow both blocks, with the g1 chain
        # hiding under the next iteration's y-g0.
        for hcq in range(2):
            for g in range(2):
                for hc in range(4 * hcq, 4 * hcq + 4):
                    for b_ in range(4):
                        b = 4 * g + b_
                        nc.tensor.matmul(
                            u_ps[g][32 * b_:32 * b_ + 16, :],
                            p_sb[:, hc, 16 * b:16 * b + 16],
                            xt16[:, b, hc, :],
                            start=False, stop=(hc == 7),
                            skip_group_check=True,
                            tile_position=(0, 32 * b_))
                if hcq == 1:
                    _act_copy(nc, u_sb[:, g, :], u_ps[g][:])
                if hcq == 0 and g == 0:
                    # PE-transpose p-h1 between the hcq0 g-blocks: its
                    # ACT evac lands during g1's matmuls, so hcq1
                    # starts with no stall
                    _p_tr(1)
        for g in range(2):
            _upd_tail(g)
    ctx.close()


_CACHE = {}


def _host_consts():
    c0 = np.zeros((128, SC, BL, 32), np.float16)
    c0[:, :, :, 0:NCAP] = np.float16(1.0 / NCAP)
    logi = np.zeros((128, SC, BL, NCAP), np.float32)
    vblk0 = np.zeros((128, OC, 128), np.float16)
    ones2 = np.zeros((128, 2), np.float16)
    ones2[0:64, 0] = 1.0
    ones2[64:128, 1] = 1.0
    o2t = np.zeros((2, 128), np.float32)
    o2t[0, 0:64] = -1.0          # sign-fix for the single-Newton rsqrt
    o2t[1, 64:128] = -1.0
    zeros = np.zeros((128, 128), np.float16)
    magic = np.full((2, 1), MAGIC, np.int32)
    ident = np.eye(128, dtype=np.float16)
    magic64 = np.full((64, 1), MAGIC, np.int32)
    return {"c0i": c0, "logi": logi, "vblki": vblk0, "ones2": ones2,
            "o2t": o2t, "zeros": zeros, "magic": magic, "ident": ident,
            "magic64": magic64}


def _get_nc():
    if "nc" not in _CACHE:
        nc = bacc.Bacc("TRN2", target_bir_lowering=False, debug=False)
        x_d = nc.dram_tensor("x16", [128, BL, SC, 1024], f16,
                             kind="ExternalInput")
        xt_d = nc.dram_tensor("xt16", [128, BL, HC, 512], f16,
                              kind="ExternalInput")
        w_d = nc.dram_tensor("w16", [128, HC, 1024], f16,
                             kind="ExternalInput")
        wt_d = nc.dram_tensor("wt16", [128, OC, 1024], f16,
                              kind="ExternalInput")
        c0_d = nc.dram_tensor("c0i", [128, SC, BL, 32], f16,
                              kind="ExternalInput")
        logits_d = nc.dram_tensor("logi", [128, SC, BL, NCAP], f32,
                                  kind="ExternalInput")
        vblk_d = nc.dram_tensor("vblki", [128, OC, 128], f16,
                                kind="ExternalInput")
        ones2_d = nc.dram_tensor("ones2", [128, 2], f16, kind="ExternalInput")
        o2t_d = nc.dram_tensor("o2t", [2, 128], f32, kind="ExternalInput")
        zeros_d = nc.dram_tensor("zeros", [128, 128], f16,
                                 kind="ExternalInput")
        magic_d = nc.dram_tensor("magic", [2, 1], i32, kind="ExternalInput")
        ident_d = nc.dram_tensor("ident", [128, 128], f16,
                                 kind="ExternalInput")
        magic64_d = nc.dram_tensor("magic64", [64, 1], i32,
                                   kind="ExternalInput")
        out_d = nc.dram_tensor("out", [128, 64], f32, kind="ExternalOutput")
        with tile.TileContext(nc) as tc:
            _build_kernel(tc, out_d.ap(), x_d.ap(), xt_d.ap(), w_d.ap(),
                          wt_d.ap(), c0_d.ap(), logits_d.ap(), vblk_d.ap(),
                          ones2_d.ap(), o2t_d.ap(), zeros_d.ap(),
                          magic_d.ap(), ident_d.ap(), magic64_d.ap())
        nc.compile()
        _CACHE["nc"] = nc
    return _CACHE["nc"]


def kernel(inputs: np.ndarray, W: np.ndarray, _trace: bool = False):
    """inputs: (512, 64, 1024) f32; W: (1, 1024, 1024) f32.
    Returns (64, 16, 64) f32."""
    nc = _get_nc()
    consts = _host_consts()
    w0 = W[0].astype(np.float16)
    w16h = np.ascontiguousarray(
        w0.reshape(HC, 128, 1024).transpose(1, 0, 2))
    wt16h = np.ascontiguousarray(
        w0.reshape(1024, OC, 128).transpose(2, 1, 0))
    x16f = inputs.astype(np.float16)              # (S, B, H)
    in_maps = []
    for c in range(N_CORES):
        xs = x16f[:, c * BL:(c + 1) * BL, :]      # (S, BL, H)
        x16h = np.ascontiguousarray(
            xs.reshape(SC, 128, BL, H).transpose(1, 2, 0, 3))
        xt16h = np.ascontiguousarray(
            xs.reshape(S, BL, HC, 128).transpose(3, 1, 2, 0))
        m = {"x16": x16h, "xt16": xt16h, "w16": w16h, "wt16": wt16h}
        m.update(consts)
        in_maps.append(m)
    kw = {}
    if _trace:
        kw = dict(trace=True, trace_cores=[0], stitch_traces=False)
    res = run_bass_kernel_spmd(nc, in_maps, core_ids=list(range(N_CORES)),
                               **kw)
    outs = []
    for c in range(N_CORES):
        v = res.results[c]["out"]          # (128=e*64+d, 64=b*8+oc)
        outs.append(v.reshape(2, 64, BL, 8).transpose(2, 3, 0, 1)
                     .reshape(BL, NCAP, DCAP))
    out = np.concatenate(outs, axis=0)
    if _trace:
        return out.astype(np.float32), res
    return out.astype(np.float32)
